# revision 1
# baseline (speedup 1.0000x reference)
"""Trainium2 Bass kernel for causal GQA attention (nn_Attention_83090437308676).

Full shapes: x [4096, 2048], 16 Q heads / 4 KV heads, d_head=128, fp32, causal,
rotary (interleaved pairs, rotary_dim=128), out = attn @ W_O + b_O.

Sharding: tensor-parallel over heads. Core c computes Q-heads {2c, 2c+1} and
KV-head c//2 (duplicated across the pair of cores sharing it), produces the
partial output z_h @ W_O_h summed over its 2 heads; the host sums the 8
partials and adds b_O.

Device-side layout trick: all matmuls contract on the partition axis, so x is
fed pre-transposed (xT [d_model, seq]) and Q/K are produced directly in
"T" layout [d_head, seq]. Scores are computed k-major (ST [k, q]) so exp(ST)
tiles serve directly as matmul operands for both the PV product (V stationary)
and the softmax denominator (all-ones stationary -> row-broadcast denominators
in PSUM), with zero on-chip transposes of the attention pattern. Rotary is
applied in a permuted head layout (even dims first) so pair elements sit in
partition halves; W_Q/W_K/b_Q/b_K are pre-permuted on the host to match.
"""

import numpy as np

SEQ = 4096
D_MODEL = 2048
D_HEAD = 128
N_HEADS = 16
N_KV = 4
N_CORES = 8
ROTARY_BASE = 10000.0
ATTN_SCALE = 11.313708498984761  # sqrt(d_head)

P = 128  # partitions
FD = 512  # matmul moving free dim / chunk width


def build_bass(seq=SEQ, d_model=D_MODEL, heads_per_core=2):
    """Emit the per-core Tile kernel. Same program for all cores (SPMD);
    per-core tensors differ only in data."""
    from contextlib import ExitStack

    import concourse.mybir as mybir
    import concourse.tile as tile
    from concourse import bacc
    from concourse.bass import ds

    f32 = mybir.dt.float32
    f32r = mybir.dt.float32r
    AF = mybir.ActivationFunctionType
    OP = mybir.AluOpType

    H = heads_per_core
    DM_TILES = d_model // P      # contraction tiles for projections
    QC = seq // FD               # 512-wide seq chunks
    MC = d_model // FD           # 512-wide output-model chunks

    nc = bacc.Bacc("TRN2", target_bir_lowering=False, debug=False,
                   num_devices=N_CORES)

    xT = nc.dram_tensor("xT", (d_model, seq), f32r, kind="ExternalInput").ap()
    wq = nc.dram_tensor("wq", (H, d_model, D_HEAD), f32r, kind="ExternalInput").ap()
    wk = nc.dram_tensor("wk", (d_model, D_HEAD), f32r, kind="ExternalInput").ap()
    wv = nc.dram_tensor("wv", (d_model, D_HEAD), f32r, kind="ExternalInput").ap()
    wo = nc.dram_tensor("wo", (H, D_HEAD, d_model), f32r, kind="ExternalInput").ap()
    bq = nc.dram_tensor("bq", (64, H, 2), f32, kind="ExternalInput").ap()
    bk = nc.dram_tensor("bk", (64, 2), f32, kind="ExternalInput").ap()
    bv = nc.dram_tensor("bv", (P, 1), f32, kind="ExternalInput").ap()
    cos2 = nc.dram_tensor("cos2", (64, seq), f32, kind="ExternalInput").ap()
    sin2 = nc.dram_tensor("sin2", (64, seq), f32, kind="ExternalInput").ap()
    ident = nc.dram_tensor("ident", (P, P), f32, kind="ExternalInput").ap()
    maskm = nc.dram_tensor("maskm", (P, P), f32r, kind="ExternalInput").ap()
    onesd = nc.dram_tensor("onesd", (P, P), f32r, kind="ExternalInput").ap()
    out = nc.dram_tensor("out", (seq, d_model), f32, kind="ExternalOutput").ap()

    with tile.TileContext(nc) as tc, ExitStack() as ctx:
        const = ctx.enter_context(tc.tile_pool(name="const", bufs=1))
        persist = ctx.enter_context(tc.tile_pool(name="persist", bufs=1))
        xt_pool = ctx.enter_context(tc.tile_pool(name="xt", bufs=17))
        qt_pool = ctx.enter_context(tc.tile_pool(name="qt", bufs=2))
        e_pool = ctx.enter_context(tc.tile_pool(name="e", bufs=4))
        wk_pool = ctx.enter_context(tc.tile_pool(name="wk", bufs=2))
        ps = ctx.enter_context(tc.tile_pool(name="ps", bufs=8, space="PSUM"))

        # ---- constants / weights resident in SBUF ----
        # Weight-chunk DMAs are interleaved with the first chunk's xt loads
        # (inside phase1(0)) so the first projection matmuls start ~2us in.
        wq_sb = const.tile([P, H, DM_TILES, D_HEAD], f32r, tag="wq")
        wk_sb = const.tile([P, DM_TILES, D_HEAD], f32r, tag="wk")
        wv_sb = const.tile([P, DM_TILES, D_HEAD], f32r, tag="wv")
        wq_r = wq.rearrange("h (t p) d -> p h t d", p=P)
        wk_r = wk.rearrange("(t p) d -> p t d", p=P)
        wv_r = wv.rearrange("(t p) d -> p t d", p=P)
        id_sb = const.tile([P, P], f32, tag="id")
        nc.sync.dma_start(id_sb[:], ident)
        mask_sb = const.tile([P, P], f32r, tag="mask")
        nc.sync.dma_start(mask_sb[:], maskm)
        bq_sb = const.tile([64, H, 2], f32, tag="bq")
        nc.sync.dma_start(bq_sb[:], bq)
        bk_sb = const.tile([64, 2], f32, tag="bk")
        nc.sync.dma_start(bk_sb[:], bk)
        bv_sb = const.tile([P, 1], f32, tag="bv")
        nc.sync.dma_start(bv_sb[:], bv)
        ones_sb = const.tile([P, P], f32r, tag="ones")
        nc.sync.dma_start(ones_sb[:], onesd)
        cos_sb = const.tile([64, seq], f32, tag="cos")
        sin_sb = const.tile([64, seq], f32, tag="sin")
        wo_sb = const.tile([P, H, d_model], f32r, tag="wo")

        # K^T (rotated) and V (natural [k, d]) for this core's KV head.
        kt_sb = persist.tile([P, seq], f32r, tag="kt")
        v_sb = persist.tile([P, seq // P, P], f32r, tag="v")

        def rotary_evac(psum, dst, b_ap, qc):
            """dst ([P, FD] slice) = rotary(psum + bias) at positions of chunk qc.

            All DVE products run at partitions 0..63 (PSUM in0 may carry a
            different base partition; two SBUF inputs may not)."""
            sl = ds(qc * FD, FD)
            x1, x2 = psum[0:64, :], psum[64:128, :]
            b1, b2 = b_ap[:, 0:1], b_ap[:, 1:2]
            t1 = wk_pool.tile([64, FD], f32, tag="rot_t1")
            t2 = wk_pool.tile([64, FD], f32, tag="rot_t2")
            t3 = wk_pool.tile([64, FD], f32, tag="rot_t3")
            t4 = wk_pool.tile([64, FD], f32, tag="rot_t4")
            nc.vector.scalar_tensor_tensor(t1[:], x1, b1, cos_sb[:, sl],
                                           op0=OP.add, op1=OP.mult)
            nc.vector.scalar_tensor_tensor(t2[:], x2, b2, sin_sb[:, sl],
                                           op0=OP.add, op1=OP.mult)
            nc.vector.scalar_tensor_tensor(t3[:], x1, b1, sin_sb[:, sl],
                                           op0=OP.add, op1=OP.mult)
            nc.vector.scalar_tensor_tensor(t4[:], x2, b2, cos_sb[:, sl],
                                           op0=OP.add, op1=OP.mult)
            # rot1 = x1 cos - x2 sin ; rot2 = x1 sin + x2 cos
            nc.vector.tensor_sub(dst[0:64, :], t1[:], t2[:])
            nc.vector.tensor_add(dst[64:128, :], t3[:], t4[:])

        def phase1(qc):
            """Q/K/V projections for seq chunk qc (two passes over resident
            xt tiles: Q heads first, then K/V -> only 2 PSUM banks at a
            time); returns the qt tile."""
            xts = [xt_pool.tile([P, FD], f32r, tag="xt", name=f"xt_{qc}_{t}")
                   for t in range(DM_TILES)]
            qp = [ps.tile([P, FD], f32, tag="ps", name=f"qp{h}_{qc}") for h in range(H)]
            for t in range(DM_TILES):
                if qc == 0:
                    nc.sync.dma_start(wq_sb[:, :, t, :], wq_r[:, :, t, :])
                nc.sync.dma_start(xts[t][:], xT[ds(t * P, P), ds(qc * FD, FD)])
                mm = dict(start=(t == 0), stop=(t == DM_TILES - 1))
                for h in range(H):
                    nc.tensor.matmul(qp[h][:], wq_sb[:, h, t, :], xts[t][:], **mm)
            if qc == 0:
                nc.sync.dma_start(cos_sb[:], cos2)
                nc.sync.dma_start(sin_sb[:], sin2)
            qt = qt_pool.tile([P, H, FD], f32r, tag="qt", name=f"qt_{qc}")
            for h in range(H):
                rotary_evac(qp[h], qt[:, h, :], bq_sb[:, h, :], qc)

            kp = ps.tile([P, FD], f32, tag="ps", name=f"kp_{qc}")
            vp = ps.tile([P, FD], f32, tag="ps", name=f"vp_{qc}")
            for t in range(DM_TILES):
                if qc == 0:
                    nc.sync.dma_start(wk_sb[:, t, :], wk_r[:, t, :])
                    nc.sync.dma_start(wv_sb[:, t, :], wv_r[:, t, :])
                mm = dict(start=(t == 0), stop=(t == DM_TILES - 1))
                nc.tensor.matmul(kp[:], wk_sb[:, t, :], xts[t][:], **mm)
                nc.tensor.matmul(vp[:], wv_sb[:, t, :], xts[t][:], **mm)
            if qc == 0:
                nc.sync.dma_start(wo_sb[:], wo.rearrange("h p m -> p h m"))
            rotary_evac(kp, kt_sb[:, ds(qc * FD, FD)], bk_sb, qc)
            # V: bias add then transpose to natural [k, d] layout
            vt = wk_pool.tile([P, FD], f32, tag="vt")
            nc.scalar.activation(vt[:], vp[:], AF.Identity, bias=bv_sb[:, 0:1])
            for j in range(FD // P):
                tp = ps.tile([P, P], f32, tag="ps", name=f"tp_{qc}_{j}")
                nc.tensor.transpose(tp[:], vt[:, ds(j * P, P)], id_sb[:])
                nc.scalar.copy(v_sb[:, qc * (FD // P) + j, :], tp[:])
            return qt

        def attention(qc, qt):
            """Causal attention for q chunk qc; returns per-head normalized z^T."""
            ztn = []
            for h in range(H):
                zt = ps.tile([P, FD], f32, tag="ps", name=f"zt_{h}_{qc}")
                den = ps.tile([P, FD], f32, tag="ps", name=f"den_{h}_{qc}")
                kt_max = 4 * qc + 3
                for kt in range(kt_max + 1):
                    o = max(0, kt * P - qc * FD)
                    n = FD - o
                    st = ps.tile([P, FD], f32, tag="ps", name=f"st_{h}_{qc}_{kt}")
                    nc.tensor.matmul(st[:, o:FD], kt_sb[:, ds(kt * P, P)],
                                     qt[:, h, o:FD], start=True, stop=True)
                    e = e_pool.tile([P, FD], f32r, tag="e", name=f"e_{h}_{qc}_{kt}")
                    nc.scalar.activation(e[:, o:FD], st[:, o:FD], AF.Exp,
                                         scale=1.0 / ATTN_SCALE)
                    if kt >= 4 * qc:  # diagonal 128-block: causal mask inside
                        nc.vector.tensor_mul(e[:, o:o + P], e[:, o:o + P], mask_sb[:])
                    acc = dict(start=(kt == 0), stop=(kt == kt_max))
                    nc.tensor.matmul(zt[:, o:FD], v_sb[:, kt, :], e[:, o:FD], **acc)
                    nc.tensor.matmul(den[0:1, o:FD], ones_sb[:, 0:1], e[:, o:FD], **acc)
                # reciprocal of one denominator row, broadcast via K=1 matmul
                rf = wk_pool.tile([1, FD], f32, tag="rf", bufs=1, name=f"rf_{h}_{qc}")
                nc.vector.reciprocal_approx_fast(rf[:], den[0:1, :])
                rr = wk_pool.tile([1, FD], f32r, tag="rr", bufs=1, name=f"rr_{h}_{qc}")
                nc.vector.tensor_scalar_mul(rr[:], rf[:], 1.0)
                # broadcast 1/den into the (already-read) den bank: saves a
                # PSUM slot so the other head's matmuls run during this chain
                nc.tensor.matmul(den[:], ones_sb[0:1, :], rr[:],
                                 start=True, stop=True)
                rden = wk_pool.tile([P, FD], f32, tag="rden", name=f"rd_{h}_{qc}")
                nc.vector.tensor_copy(rden[:], den[:])
                z = wk_pool.tile([P, FD], f32r, tag="ztn", bufs=3, name=f"z_{h}_{qc}")
                nc.vector.tensor_mul(z[:], zt[:], rden[:])
                ztn.append(z)
            return ztn

        def outproj(qc, ztn):
            for sub in range(FD // P):
                for mc in range(MC):
                    op_ps = ps.tile([P, FD], f32, tag="ps", name=f"op_{qc}_{sub}_{mc}")
                    for h in range(H):
                        nc.tensor.matmul(op_ps[:], ztn[h][:, ds(sub * P, P)],
                                         wo_sb[:, h, ds(mc * FD, FD)],
                                         start=(h == 0), stop=(h == H - 1))
                    ot = wk_pool.tile([P, FD], f32, tag="ot", bufs=2,
                                      name=f"ot_{qc}_{sub}_{mc}")
                    nc.scalar.copy(ot[:], op_ps[:])
                    nc.sync.dma_start(out[ds(qc * FD + sub * P, P), ds(mc * FD, FD)],
                                      ot[:])

        # Software pipeline: projections for chunk qc+1 are emitted before
        # attention of chunk qc so the PE always has runnable matmuls while
        # attention waits on softmax chains.
        qts = {0: phase1(0)}
        for qc in range(QC):
            if qc + 1 < QC:
                qts[qc + 1] = phase1(qc + 1)
            ztn = attention(qc, qts.pop(qc))
            outproj(qc, ztn)
    nc.compile()
    return nc


_PERM = None


def _perm():
    global _PERM
    if _PERM is None:
        _PERM = np.concatenate([np.arange(0, D_HEAD, 2), np.arange(1, D_HEAD, 2)])
    return _PERM


def host_inputs(x, W_Q, W_K, W_V, W_O, b_Q, b_K, b_V, core,
                heads_per_core=2):
    """Build the per-core input map (numpy, named as in build_bass)."""
    seq = x.shape[0]
    perm = _perm()
    h0 = core * heads_per_core
    kv = h0 // (N_HEADS // N_KV)
    pairs = D_HEAD // 2
    freqs = 1.0 / ROTARY_BASE ** (np.arange(pairs, dtype=np.float64) / pairs)
    ang = np.outer(np.arange(seq), freqs)  # [seq, 64]
    cos = np.cos(ang).T.astype(np.float32)  # [64, seq]
    sin = np.sin(ang).T.astype(np.float32)
    return {
        "xT": np.ascontiguousarray(x.T),
        "wq": np.ascontiguousarray(W_Q[h0:h0 + heads_per_core][:, :, perm]),
        "wk": np.ascontiguousarray(W_K[kv][:, perm]),
        "wv": np.ascontiguousarray(W_V[kv]),
        "wo": np.ascontiguousarray(W_O[h0:h0 + heads_per_core]),
        "bq": np.ascontiguousarray(
            b_Q[h0:h0 + heads_per_core][:, perm]
            .reshape(heads_per_core, 2, 64).transpose(2, 0, 1)),
        "bk": np.ascontiguousarray(b_K[kv][perm].reshape(2, 64).T),
        "bv": np.ascontiguousarray(b_V[kv][:, None]),
        "cos2": cos,
        "sin2": sin,
        "ident": np.eye(P, dtype=np.float32),
        "maskm": np.triu(np.ones((P, P), dtype=np.float32)),
        "onesd": np.ones((P, P), dtype=np.float32),
    }


_NC_CACHE = {}


def kernel(x, W_Q, W_K, W_V, W_O, b_Q, b_K, b_V, b_O):
    import sys
    if "/opt/trn_rl_repo" not in sys.path:
        sys.path.insert(0, "/opt/trn_rl_repo")
    from concourse import bass_utils

    x = np.asarray(x, dtype=np.float32)
    key = (x.shape[0], x.shape[1])
    if key not in _NC_CACHE:
        _NC_CACHE[key] = build_bass(seq=x.shape[0], d_model=x.shape[1])
    nc = _NC_CACHE[key]

    in_maps = [
        host_inputs(x, np.asarray(W_Q, np.float32), np.asarray(W_K, np.float32),
                    np.asarray(W_V, np.float32), np.asarray(W_O, np.float32),
                    np.asarray(b_Q, np.float32), np.asarray(b_K, np.float32),
                    np.asarray(b_V, np.float32), core)
        for core in range(N_CORES)
    ]
    res = bass_utils.run_bass_kernel_spmd(nc, in_maps, core_ids=list(range(N_CORES)))
    total = np.zeros((x.shape[0], x.shape[1]), dtype=np.float32)
    for r in res.results:
        total += r["out"]
    total += np.asarray(b_O, np.float32)[None, :]
    return total



# revision 7
# speedup vs baseline: 1.2748x; 1.2748x over previous
"""Trainium2 Bass kernel for causal GQA attention (nn_Attention_83090437308676).

Full shapes: x [4096, 2048], 16 Q heads / 4 KV heads, d_head=128, fp32, causal,
rotary (interleaved pairs, rotary_dim=128), out = attn @ W_O + b_O.

Sharding: tensor-parallel over heads. Core c computes Q-heads {2c, 2c+1} and
KV-head c//2 (duplicated across the pair of cores sharing it), produces the
partial output z_h @ W_O_h summed over its 2 heads; the host sums the 8
partials (bf16) in fp32 and adds b_O.

v2 vs baseline (615us):
 - all matmul operands bf16 (fp32 PSUM accumulation stays) -> ~2x PE rate; the
   HW runs f32r matmuls at ~1.3GHz effective vs ~2.4GHz for bf16.
 - denominator: instead of a ones-stationary matmul per (head, kt) tile (a
   full extra pass of e through the PE), e tiles are accumulated on the DVE
   into a per-chunk esum [128, 2*FD]; one ones-matmul per (chunk, head)
   contracts the final 128 k-rows.
 - both heads merged per kt step: scores land in one [128, 1024] PSUM tile
   (2 banks), a single Exp instruction evacuates both heads (halves the Act
   engine's fixed per-instruction overhead).
 - explicit software pipelining: K/V projection of chunk qc+1 and Q projection
   of chunk qc+2 are emitted as "filler" slices between attention kt steps so
   the PE never waits on the softmax (Act) chain; x tiles for chunk qc+2 are
   DMA-prefetched at the start of attention(qc).
"""

from collections import deque

import numpy as np

SEQ = 4096
D_MODEL = 2048
D_HEAD = 128
N_HEADS = 16
N_KV = 4
N_CORES = 8
ROTARY_BASE = 10000.0
ATTN_SCALE = 11.313708498984761  # sqrt(d_head)

P = 128  # partitions
FD = 512  # matmul moving free dim / chunk width


def build_bass(seq=SEQ, d_model=D_MODEL, heads_per_core=2):
    """Emit the per-core Tile kernel. Same program for all cores (SPMD);
    per-core tensors differ only in data."""
    from contextlib import ExitStack

    import concourse.mybir as mybir
    import concourse.tile as tile
    from concourse import bacc
    from concourse.bass import ds

    f32 = mybir.dt.float32
    bf16 = mybir.dt.bfloat16
    AF = mybir.ActivationFunctionType
    OP = mybir.AluOpType

    H = heads_per_core
    DM_TILES = d_model // P      # contraction tiles for projections
    QC = seq // FD               # 512-wide seq chunks
    W2 = 2 * FD                  # merged two-head tile width

    nc = bacc.Bacc("TRN2", target_bir_lowering=False, debug=False,
                   num_devices=N_CORES)

    xT = nc.dram_tensor("xT", (d_model, seq), bf16, kind="ExternalInput").ap()
    wq = nc.dram_tensor("wq", (H, d_model, D_HEAD), bf16, kind="ExternalInput").ap()
    wk = nc.dram_tensor("wk", (d_model, D_HEAD), bf16, kind="ExternalInput").ap()
    wv = nc.dram_tensor("wv", (d_model, D_HEAD), bf16, kind="ExternalInput").ap()
    wo = nc.dram_tensor("wo", (H, D_HEAD, d_model), bf16, kind="ExternalInput").ap()
    bq = nc.dram_tensor("bq", (64, H, 2), f32, kind="ExternalInput").ap()
    bk = nc.dram_tensor("bk", (64, 2), f32, kind="ExternalInput").ap()
    bv = nc.dram_tensor("bv", (P, 1), f32, kind="ExternalInput").ap()
    cos2 = nc.dram_tensor("cos2", (64, seq), f32, kind="ExternalInput").ap()
    sin2 = nc.dram_tensor("sin2", (64, seq), f32, kind="ExternalInput").ap()
    ident = nc.dram_tensor("ident", (P, P), f32, kind="ExternalInput").ap()
    maskm = nc.dram_tensor("maskm", (P, P), bf16, kind="ExternalInput").ap()
    onesd = nc.dram_tensor("onesd", (P, P), bf16, kind="ExternalInput").ap()
    out = nc.dram_tensor("out", (seq, d_model), bf16, kind="ExternalOutput").ap()

    with tile.TileContext(nc) as tc, ExitStack() as ctx:
        const = ctx.enter_context(tc.tile_pool(name="const", bufs=1))
        persist = ctx.enter_context(tc.tile_pool(name="persist", bufs=1))
        xt_pool = ctx.enter_context(tc.tile_pool(name="xt", bufs=34))
        qt_pool = ctx.enter_context(tc.tile_pool(name="qt", bufs=3))
        e_pool = ctx.enter_context(tc.tile_pool(name="e", bufs=3))
        sb = ctx.enter_context(tc.tile_pool(name="sb", bufs=2))
        # PSUM: big pool = 2 x [128,1024] (4 banks): stm / den / rden / op
        #       zt pool  = 1 x [128,1024] (2 banks): per-chunk PV accumulator
        #       acc pool = 2 x [128,512]  (2 banks): qp pair / kp+vp / tp
        psB = ctx.enter_context(tc.tile_pool(name="psB", bufs=2, space="PSUM"))
        psZ = ctx.enter_context(tc.tile_pool(name="psZ", bufs=1, space="PSUM"))
        psA = ctx.enter_context(tc.tile_pool(name="psA", bufs=2, space="PSUM"))

        # ---- constants / weights resident in SBUF ----
        wq_sb = const.tile([P, H, DM_TILES, D_HEAD], bf16, tag="wq")
        wk_sb = const.tile([P, DM_TILES, D_HEAD], bf16, tag="wk")
        wv_sb = const.tile([P, DM_TILES, D_HEAD], bf16, tag="wv")
        wq_r = wq.rearrange("h (t p) d -> p h t d", p=P)
        wk_r = wk.rearrange("(t p) d -> p t d", p=P)
        wv_r = wv.rearrange("(t p) d -> p t d", p=P)
        id_sb = const.tile([P, P], f32, tag="id")
        nc.sync.dma_start(id_sb[:], ident)
        mask_sb = const.tile([P, P], bf16, tag="mask")
        nc.sync.dma_start(mask_sb[:], maskm)
        bq_sb = const.tile([64, H, 2], f32, tag="bq")
        nc.sync.dma_start(bq_sb[:], bq)
        bk_sb = const.tile([64, 2], f32, tag="bk")
        nc.sync.dma_start(bk_sb[:], bk)
        bv_sb = const.tile([P, 1], f32, tag="bv")
        nc.sync.dma_start(bv_sb[:], bv)
        ones_sb = const.tile([P, P], bf16, tag="ones")
        nc.sync.dma_start(ones_sb[:], onesd)
        cos_sb = const.tile([64, seq], f32, tag="cos")
        sin_sb = const.tile([64, seq], f32, tag="sin")
        wo_sb = const.tile([P, H, d_model], bf16, tag="wo")

        # K^T (rotated) and V (natural [k, d]) for this core's KV head.
        kt_sb = persist.tile([P, seq], bf16, tag="kt")
        v_sb = persist.tile([P, seq // P, P], bf16, tag="v")

        qts = {}       # qc -> qt tile
        xts_map = {}   # qc -> list of xt tiles (DMA prefetched)
        ztn_map = {}   # qc -> normalized z (bf16, [P, W2])

        def rotary_evac(psum, dst, b_ap, qc):
            """dst ([P, FD] slice, bf16) = rotary(psum + bias) at chunk qc."""
            sl = ds(qc * FD, FD)
            x1, x2 = psum[0:64, :], psum[64:128, :]
            b1, b2 = b_ap[:, 0:1], b_ap[:, 1:2]
            t1 = sb.tile([64, FD], f32, tag="rot_t1")
            t2 = sb.tile([64, FD], f32, tag="rot_t2")
            t3 = sb.tile([64, FD], f32, tag="rot_t3")
            t4 = sb.tile([64, FD], f32, tag="rot_t4")
            nc.vector.scalar_tensor_tensor(t1[:], x1, b1, cos_sb[:, sl],
                                           op0=OP.add, op1=OP.mult)
            nc.vector.scalar_tensor_tensor(t2[:], x2, b2, sin_sb[:, sl],
                                           op0=OP.add, op1=OP.mult)
            nc.vector.scalar_tensor_tensor(t3[:], x1, b1, sin_sb[:, sl],
                                           op0=OP.add, op1=OP.mult)
            nc.vector.scalar_tensor_tensor(t4[:], x2, b2, cos_sb[:, sl],
                                           op0=OP.add, op1=OP.mult)
            nc.vector.tensor_sub(dst[0:64, :], t1[:], t2[:])
            nc.vector.tensor_add(dst[64:128, :], t3[:], t4[:])

        def load_x(qc):
            """DMA-prefetch the 16 x tiles for chunk qc."""
            xts = [xt_pool.tile([P, FD], bf16, tag="xt", name=f"xt_{qc}_{t}")
                   for t in range(DM_TILES)]
            for t in range(DM_TILES):
                nc.sync.dma_start(xts[t][:], xT[ds(t * P, P), ds(qc * FD, FD)])
            xts_map[qc] = xts

        def projQ_gen(qc):
            """Q projection + rotary for chunk qc (xts already prefetched)."""
            xts = xts_map[qc]
            qp = [psA.tile([P, FD], f32, tag="a", name=f"qp{h}_{qc}")
                  for h in range(H)]
            for t in range(DM_TILES):
                if qc == 0:
                    nc.sync.dma_start(wq_sb[:, :, t, :], wq_r[:, :, t, :])
                mm = dict(start=(t == 0), stop=(t == DM_TILES - 1))
                for h in range(H):
                    nc.tensor.matmul(qp[h][:], wq_sb[:, h, t, :], xts[t][:], **mm)
                yield
            if qc == 0:
                nc.sync.dma_start(cos_sb[:], cos2)
                nc.sync.dma_start(sin_sb[:], sin2)
            qt = qt_pool.tile([P, H, FD], bf16, tag="qt", name=f"qt_{qc}")
            for h in range(H):
                rotary_evac(qp[h], qt[:, h, :], bq_sb[:, h, :], qc)
            qts[qc] = qt
            yield

        def projKV_gen(qc):
            """K/V projection for chunk qc: K rotary -> kt_sb, V -> v_sb."""
            xts = xts_map[qc]
            kp = psA.tile([P, FD], f32, tag="a", name=f"kp_{qc}")
            vp = psA.tile([P, FD], f32, tag="a", name=f"vp_{qc}")
            for t in range(DM_TILES):
                if qc == 0:
                    nc.sync.dma_start(wk_sb[:, t, :], wk_r[:, t, :])
                    nc.sync.dma_start(wv_sb[:, t, :], wv_r[:, t, :])
                mm = dict(start=(t == 0), stop=(t == DM_TILES - 1))
                nc.tensor.matmul(kp[:], wk_sb[:, t, :], xts[t][:], **mm)
                nc.tensor.matmul(vp[:], wv_sb[:, t, :], xts[t][:], **mm)
                yield
            if qc == 0:
                nc.sync.dma_start(wo_sb[:], wo.rearrange("h p m -> p h m"))
            rotary_evac(kp, kt_sb[:, ds(qc * FD, FD)], bk_sb, qc)
            yield
            # V: bias add then transpose to natural [k, d] layout (f32 through
            # the PE transpose; cast to bf16 on the PSUM->v_sb copy)
            vt = sb.tile([P, FD], f32, tag="vt")
            nc.scalar.activation(vt[:], vp[:], AF.Identity, bias=bv_sb[:, 0:1])
            tp = psA.tile([P, FD], f32, tag="a", name=f"tp_{qc}")
            for j in range(FD // P):
                nc.tensor.transpose(tp[:, ds(j * P, P)], vt[:, ds(j * P, P)],
                                    id_sb[:])
            nc.scalar.copy(v_sb[:, qc * (FD // P):(qc + 1) * (FD // P), :], tp[:])
            yield

        def pull(fillers, k):
            while k > 0 and fillers:
                try:
                    next(fillers[0])
                    k -= 1
                except StopIteration:
                    fillers.popleft()

        def attention(qc, fillers):
            """Causal attention for q chunk qc, both heads per kt step."""
            qt = qts.pop(qc)
            zt = psZ.tile([P, W2], f32, tag="z", name=f"zt_{qc}")
            esum = sb.tile([P, W2], bf16, tag="esum", name=f"esum_{qc}")
            kt_max = 4 * qc + 3
            for kt in range(kt_max + 1):
                o = max(0, kt * P - qc * FD)
                stm = psB.tile([P, W2], f32, tag="B", name=f"stm_{qc}_{kt}")
                nc.tensor.matmul(stm[:, o:FD], kt_sb[:, ds(kt * P, P)],
                                 qt[:, 0, o:FD], start=True, stop=True)
                nc.tensor.matmul(stm[:, FD + o:W2], kt_sb[:, ds(kt * P, P)],
                                 qt[:, 1, o:FD], start=True, stop=True)
                e = e_pool.tile([P, W2], bf16, tag="e", name=f"e_{qc}_{kt}")
                nc.scalar.activation(e[:, o:W2], stm[:, o:W2], AF.Exp,
                                     scale=1.0 / ATTN_SCALE)
                if kt >= 4 * qc:  # diagonal 128-block: causal mask inside
                    nc.vector.tensor_mul(e[:, o:o + P], e[:, o:o + P], mask_sb[:])
                    nc.vector.tensor_mul(e[:, FD + o:FD + o + P],
                                         e[:, FD + o:FD + o + P], mask_sb[:])
                if kt == 0:
                    nc.vector.tensor_copy(esum[:], e[:])
                elif o == 0:
                    nc.vector.tensor_add(esum[:], esum[:], e[:])
                else:
                    nc.vector.tensor_add(esum[:, o:FD], esum[:, o:FD],
                                         e[:, o:FD])
                    nc.vector.tensor_add(esum[:, FD + o:W2], esum[:, FD + o:W2],
                                         e[:, FD + o:W2])
                acc = dict(start=(kt == 0), stop=(kt == kt_max))
                nc.tensor.matmul(zt[:, o:FD], v_sb[:, kt, :], e[:, o:FD], **acc)
                nc.tensor.matmul(zt[:, FD + o:W2], v_sb[:, kt, :],
                                 e[:, FD + o:W2], **acc)
                pull(fillers, 2)
            # denominator: contract esum's 128 k-rows with a ones column
            den = psB.tile([P, W2], f32, tag="B", name=f"den_{qc}")
            nc.tensor.matmul(den[0:1, 0:FD], ones_sb[:, 0:1], esum[:, 0:FD],
                             start=True, stop=True)
            nc.tensor.matmul(den[0:1, FD:W2], ones_sb[:, 0:1], esum[:, FD:W2],
                             start=True, stop=True)
            pull(fillers, 3)
            rf = sb.tile([1, W2], f32, tag="rf", name=f"rf_{qc}")
            nc.vector.reciprocal_approx_fast(rf[:], den[0:1, :])
            rr = sb.tile([1, W2], bf16, tag="rr", name=f"rr_{qc}")
            nc.vector.tensor_scalar_mul(rr[:], rf[:], 1.0)
            rden_ps = psB.tile([P, W2], f32, tag="B", name=f"rden_{qc}")
            nc.tensor.matmul(rden_ps[:, 0:FD], ones_sb[0:1, :], rr[0:1, 0:FD],
                             start=True, stop=True)
            nc.tensor.matmul(rden_ps[:, FD:W2], ones_sb[0:1, :], rr[0:1, FD:W2],
                             start=True, stop=True)
            pull(fillers, len(fillers) + 40)  # drain remaining fillers
            rden = sb.tile([P, W2], f32, tag="rden", name=f"rd_{qc}")
            nc.vector.tensor_copy(rden[:], rden_ps[:])
            ztn = sb.tile([P, W2], bf16, tag="ztn", name=f"z_{qc}")
            nc.vector.tensor_mul(ztn[:], zt[:], rden[:])
            ztn_map[qc] = ztn

        def outproj(qc):
            ztn = ztn_map.pop(qc)
            for sub in range(FD // P):
                for mcp in range(2):
                    op_ps = psB.tile([P, W2], f32, tag="B",
                                     name=f"op_{qc}_{sub}_{mcp}")
                    for half in range(2):
                        mc = mcp * 2 + half
                        for h in range(H):
                            nc.tensor.matmul(
                                op_ps[:, half * FD:(half + 1) * FD],
                                ztn[:, h * FD + sub * P:h * FD + sub * P + P],
                                wo_sb[:, h, ds(mc * FD, FD)],
                                start=(h == 0), stop=(h == H - 1))
                    ot = sb.tile([P, W2], bf16, tag="ot",
                                 name=f"ot_{qc}_{sub}_{mcp}")
                    if (sub + mcp) % 2 == 0:
                        nc.scalar.copy(ot[:], op_ps[:])
                    else:
                        nc.vector.tensor_copy(ot[:], op_ps[:])
                    nc.sync.dma_start(
                        out[ds(qc * FD + sub * P, P), ds(mcp * W2, W2)], ot[:])

        # ---- schedule ----
        load_x(0)
        for _ in projQ_gen(0):
            pass
        load_x(1)
        for _ in projKV_gen(0):
            pass
        for _ in projQ_gen(1):
            pass
        for qc in range(QC):
            if qc + 2 < QC:
                load_x(qc + 2)
            fillers = deque()
            if qc + 1 < QC:
                fillers.append(projKV_gen(qc + 1))
            if qc + 2 < QC:
                fillers.append(projQ_gen(qc + 2))
            attention(qc, fillers)
            outproj(qc)
    nc.compile()
    return nc


_PERM = None


def _perm():
    global _PERM
    if _PERM is None:
        _PERM = np.concatenate([np.arange(0, D_HEAD, 2), np.arange(1, D_HEAD, 2)])
    return _PERM


def host_inputs(x, W_Q, W_K, W_V, W_O, b_Q, b_K, b_V, core,
                heads_per_core=2):
    """Build the per-core input map (numpy, named as in build_bass)."""
    import ml_dtypes
    bf16 = ml_dtypes.bfloat16
    seq = x.shape[0]
    perm = _perm()
    h0 = core * heads_per_core
    kv = h0 // (N_HEADS // N_KV)
    pairs = D_HEAD // 2
    freqs = 1.0 / ROTARY_BASE ** (np.arange(pairs, dtype=np.float64) / pairs)
    ang = np.outer(np.arange(seq), freqs)  # [seq, 64]
    cos = np.cos(ang).T.astype(np.float32)  # [64, seq]
    sin = np.sin(ang).T.astype(np.float32)
    return {
        "xT": np.ascontiguousarray(x.T).astype(bf16),
        "wq": np.ascontiguousarray(W_Q[h0:h0 + heads_per_core][:, :, perm]).astype(bf16),
        "wk": np.ascontiguousarray(W_K[kv][:, perm]).astype(bf16),
        "wv": np.ascontiguousarray(W_V[kv]).astype(bf16),
        "wo": np.ascontiguousarray(W_O[h0:h0 + heads_per_core]).astype(bf16),
        "bq": np.ascontiguousarray(
            b_Q[h0:h0 + heads_per_core][:, perm]
            .reshape(heads_per_core, 2, 64).transpose(2, 0, 1)),
        "bk": np.ascontiguousarray(b_K[kv][perm].reshape(2, 64).T),
        "bv": np.ascontiguousarray(b_V[kv][:, None]),
        "cos2": cos,
        "sin2": sin,
        "ident": np.eye(P, dtype=np.float32),
        "maskm": np.triu(np.ones((P, P), dtype=np.float32)).astype(bf16),
        "onesd": np.ones((P, P), dtype=np.float32).astype(bf16),
    }


_NC_CACHE = {}


def kernel(x, W_Q, W_K, W_V, W_O, b_Q, b_K, b_V, b_O):
    import sys
    if "/opt/trn_rl_repo" not in sys.path:
        sys.path.insert(0, "/opt/trn_rl_repo")
    from concourse import bass_utils

    x = np.asarray(x, dtype=np.float32)
    key = (x.shape[0], x.shape[1])
    if key not in _NC_CACHE:
        _NC_CACHE[key] = build_bass(seq=x.shape[0], d_model=x.shape[1])
    nc = _NC_CACHE[key]

    in_maps = [
        host_inputs(x, np.asarray(W_Q, np.float32), np.asarray(W_K, np.float32),
                    np.asarray(W_V, np.float32), np.asarray(W_O, np.float32),
                    np.asarray(b_Q, np.float32), np.asarray(b_K, np.float32),
                    np.asarray(b_V, np.float32), core)
        for core in range(N_CORES)
    ]
    res = bass_utils.run_bass_kernel_spmd(nc, in_maps, core_ids=list(range(N_CORES)))
    total = np.zeros((x.shape[0], x.shape[1]), dtype=np.float32)
    for r in res.results:
        total += np.asarray(r["out"], dtype=np.float32)
    total += np.asarray(b_O, np.float32)[None, :]
    return total


# revision 22
# speedup vs baseline: 1.4422x; 1.1313x over previous
"""Trainium2 Bass kernel for causal GQA attention (nn_Attention_83090437308676).

Full shapes: x [4096, 2048], 16 Q heads / 4 KV heads, d_head=128, fp32, causal,
rotary (interleaved pairs, rotary_dim=128), out = attn @ W_O + b_O.

Sharding: tensor-parallel over heads. Core c computes Q-heads {2c, 2c+1} and
KV-head c//2 (duplicated across the pair of cores sharing it), produces the
partial output z_h @ W_O_h summed over its 2 heads; the host sums the 8
partials (bf16) in fp32 and adds b_O.

v2 vs baseline (615us):
 - all matmul operands bf16 (fp32 PSUM accumulation stays) -> ~2x PE rate; the
   HW runs f32r matmuls at ~1.3GHz effective vs ~2.4GHz for bf16.
 - denominator: instead of a ones-stationary matmul per (head, kt) tile (a
   full extra pass of e through the PE), e tiles are accumulated on the DVE
   into a per-chunk esum [128, 2*FD]; one ones-matmul per (chunk, head)
   contracts the final 128 k-rows.
 - both heads merged per kt step: scores land in one [128, 1024] PSUM tile
   (2 banks), a single Exp instruction evacuates both heads (halves the Act
   engine's fixed per-instruction overhead).
 - explicit software pipelining: K/V projection of chunk qc+1 and Q projection
   of chunk qc+2 are emitted as "filler" slices between attention kt steps so
   the PE never waits on the softmax (Act) chain; x tiles for chunk qc+2 are
   DMA-prefetched at the start of attention(qc).
"""

from collections import deque

import numpy as np

SEQ = 4096
D_MODEL = 2048
D_HEAD = 128
N_HEADS = 16
N_KV = 4
N_CORES = 8
ROTARY_BASE = 10000.0
ATTN_SCALE = 11.313708498984761  # sqrt(d_head)

P = 128  # partitions
FD = 512  # matmul moving free dim / chunk width


def build_bass(seq=SEQ, d_model=D_MODEL, heads_per_core=2):
    """Emit the per-core Tile kernel. Same program for all cores (SPMD);
    per-core tensors differ only in data."""
    from contextlib import ExitStack

    import concourse.mybir as mybir
    import concourse.tile as tile
    from concourse import bacc
    from concourse.bass import ds

    f32 = mybir.dt.float32
    bf16 = mybir.dt.bfloat16
    AF = mybir.ActivationFunctionType
    OP = mybir.AluOpType

    H = heads_per_core
    DM_TILES = d_model // P      # contraction tiles for projections
    QC = seq // FD               # 512-wide seq chunks
    W2 = 2 * FD                  # merged two-head tile width

    nc = bacc.Bacc("TRN2", target_bir_lowering=False, debug=False,
                   num_devices=N_CORES)

    xT = nc.dram_tensor("xT", (d_model, seq), bf16, kind="ExternalInput").ap()
    wq = nc.dram_tensor("wq", (H, d_model, D_HEAD), bf16, kind="ExternalInput").ap()
    wk = nc.dram_tensor("wk", (d_model, D_HEAD), bf16, kind="ExternalInput").ap()
    wv = nc.dram_tensor("wv", (d_model, D_HEAD), bf16, kind="ExternalInput").ap()
    wo = nc.dram_tensor("wo", (H, D_HEAD, d_model), bf16, kind="ExternalInput").ap()
    bq = nc.dram_tensor("bq", (64, H, 2), f32, kind="ExternalInput").ap()
    bk = nc.dram_tensor("bk", (64, 2), f32, kind="ExternalInput").ap()
    bv = nc.dram_tensor("bv", (P, 1), f32, kind="ExternalInput").ap()
    cos2 = nc.dram_tensor("cos2", (64, seq), bf16, kind="ExternalInput").ap()
    sin2 = nc.dram_tensor("sin2", (64, seq), bf16, kind="ExternalInput").ap()
    ident = nc.dram_tensor("ident", (P, P), f32, kind="ExternalInput").ap()
    maskm = nc.dram_tensor("maskm", (P, P), bf16, kind="ExternalInput").ap()
    onesd = nc.dram_tensor("onesd", (P, P), bf16, kind="ExternalInput").ap()
    out = nc.dram_tensor("out", (seq, d_model), bf16, kind="ExternalOutput").ap()

    with tile.TileContext(nc) as tc, ExitStack() as ctx:
        const = ctx.enter_context(tc.tile_pool(name="const", bufs=1))
        persist = ctx.enter_context(tc.tile_pool(name="persist", bufs=1))
        xt_pool = ctx.enter_context(tc.tile_pool(name="xt", bufs=50))
        qt_pool = ctx.enter_context(tc.tile_pool(name="qt", bufs=3))
        e_pool = ctx.enter_context(tc.tile_pool(name="e", bufs=3))
        sb = ctx.enter_context(tc.tile_pool(name="sb", bufs=2))
        # PSUM: big pool = 2 x [128,1024] (4 banks): stm / den / rden / op
        #       zt pool  = 1 x [128,1024] (2 banks): per-chunk PV accumulator
        #       acc pool = 2 x [128,512]  (2 banks): qp pair / kp+vp / tp
        psB = ctx.enter_context(tc.tile_pool(name="psB", bufs=2, space="PSUM"))
        psZ = ctx.enter_context(tc.tile_pool(name="psZ", bufs=1, space="PSUM"))
        psA = ctx.enter_context(tc.tile_pool(name="psA", bufs=2, space="PSUM"))

        # ---- constants / weights resident in SBUF ----
        # cos/sin first: the chunk-0 rotary (critical path to the first
        # attention matmul) waits on them.
        cos_sb = const.tile([64, seq], bf16, tag="cos")
        sin_sb = const.tile([64, seq], bf16, tag="sin")
        nc.sync.dma_start(cos_sb[:], cos2)
        nc.sync.dma_start(sin_sb[:], sin2)
        wq_sb = const.tile([P, H, DM_TILES, D_HEAD], bf16, tag="wq")
        wk_sb = const.tile([P, DM_TILES, D_HEAD], bf16, tag="wk")
        wv_sb = const.tile([P, DM_TILES, D_HEAD], bf16, tag="wv")
        wq_r = wq.rearrange("h (t p) d -> p h t d", p=P)
        wk_r = wk.rearrange("(t p) d -> p t d", p=P)
        wv_r = wv.rearrange("(t p) d -> p t d", p=P)
        id_sb = const.tile([P, P], f32, tag="id")
        nc.sync.dma_start(id_sb[:], ident)
        mask_sb = const.tile([P, P], bf16, tag="mask")
        nc.sync.dma_start(mask_sb[:], maskm)
        bq_sb = const.tile([64, H, 2], f32, tag="bq")
        nc.sync.dma_start(bq_sb[:], bq)
        bk_sb = const.tile([64, 2], f32, tag="bk")
        nc.sync.dma_start(bk_sb[:], bk)
        bv_sb = const.tile([P, 1], f32, tag="bv")
        nc.sync.dma_start(bv_sb[:], bv)
        ones_sb = const.tile([P, P], bf16, tag="ones")
        nc.sync.dma_start(ones_sb[:], onesd)
        wo_sb = const.tile([P, H, d_model], bf16, tag="wo")

        # K^T (rotated) and V (natural [k, d]) for this core's KV head.
        kt_sb = persist.tile([P, seq], bf16, tag="kt")
        v_sb = persist.tile([P, seq // P, P], bf16, tag="v")

        qts = {}       # qc -> qt tile
        xts_map = {}   # qc -> list of xt tiles (DMA prefetched)
        ztn_map = {}   # qc -> normalized z (bf16, [P, W2])

        def rotary_evac(psum, dst, b_ap, qc):
            """dst ([P, FD] slice, bf16) = rotary(psum + bias) at chunk qc.

            The bias add + bf16 cast goes through the Act engine (per-partition
            bias); the cos/sin algebra then runs all-bf16 on the DVE at double
            rate (the [64, x] ops only use half the lanes, so halving the
            element cost matters)."""
            sl = ds(qc * FD, FD)
            x1 = sb.tile([64, FD], bf16, tag="qsb1")
            x2 = sb.tile([64, FD], bf16, tag="qsb2")
            nc.scalar.activation(x1[:], psum[0:64, :], AF.Identity,
                                 bias=b_ap[:, 0:1])
            nc.scalar.activation(x2[:], psum[64:128, :], AF.Identity,
                                 bias=b_ap[:, 1:2])
            x1, x2 = x1[:], x2[:]
            t1 = sb.tile([64, FD], bf16, tag="rot_t1")
            t2 = sb.tile([64, FD], bf16, tag="rot_t2")
            t3 = sb.tile([64, FD], bf16, tag="rot_t3")
            t4 = sb.tile([64, FD], bf16, tag="rot_t4")
            nc.vector.tensor_mul(t1[:], x1, cos_sb[:, sl])
            nc.vector.tensor_mul(t2[:], x2, sin_sb[:, sl])
            nc.vector.tensor_mul(t3[:], x1, sin_sb[:, sl])
            nc.vector.tensor_mul(t4[:], x2, cos_sb[:, sl])
            nc.vector.tensor_sub(dst[0:64, :], t1[:], t2[:])
            nc.vector.tensor_add(dst[64:128, :], t3[:], t4[:])

        def load_x(qc):
            """DMA-prefetch the 16 x tiles for chunk qc."""
            xts = [xt_pool.tile([P, FD], bf16, tag="xt", name=f"xt_{qc}_{t}")
                   for t in range(DM_TILES)]
            for t in range(DM_TILES):
                nc.sync.dma_start(xts[t][:], xT[ds(t * P, P), ds(qc * FD, FD)])
            xts_map[qc] = xts

        def projQ_gen(qc):
            """Q projection + rotary for chunk qc (xts already prefetched)."""
            xts = xts_map[qc]
            qp = [psA.tile([P, FD], f32, tag="a", name=f"qp{h}_{qc}")
                  for h in range(H)]
            for t in range(DM_TILES):
                if qc == 0:
                    nc.sync.dma_start(wq_sb[:, :, t, :], wq_r[:, :, t, :])
                mm = dict(start=(t == 0), stop=(t == DM_TILES - 1))
                for h in range(H):
                    nc.tensor.matmul(qp[h][:], wq_sb[:, h, t, :], xts[t][:], **mm)
                yield
            qt = qt_pool.tile([P, H, FD], bf16, tag="qt", name=f"qt_{qc}")
            for h in range(H):
                rotary_evac(qp[h], qt[:, h, :], bq_sb[:, h, :], qc)
            qts[qc] = qt
            yield

        def projKV_gen(qc):
            """K/V projection for chunk qc: K rotary -> kt_sb, V -> v_sb."""
            xts = xts_map[qc]
            kp = psA.tile([P, FD], f32, tag="a", name=f"kp_{qc}")
            vp = psA.tile([P, FD], f32, tag="a", name=f"vp_{qc}")
            for t in range(DM_TILES):
                if qc == 0:
                    nc.sync.dma_start(wk_sb[:, t, :], wk_r[:, t, :])
                    nc.sync.dma_start(wv_sb[:, t, :], wv_r[:, t, :])
                mm = dict(start=(t == 0), stop=(t == DM_TILES - 1))
                nc.tensor.matmul(kp[:], wk_sb[:, t, :], xts[t][:], **mm)
                nc.tensor.matmul(vp[:], wv_sb[:, t, :], xts[t][:], **mm)
                yield
            if qc == 0:
                nc.sync.dma_start(wo_sb[:], wo.rearrange("h p m -> p h m"))
            rotary_evac(kp, kt_sb[:, ds(qc * FD, FD)], bk_sb, qc)
            yield
            # V: bias add then transpose to natural [k, d] layout (f32 through
            # the PE transpose; cast to bf16 on the PSUM->v_sb copy)
            vt = sb.tile([P, FD], f32, tag="vt")
            nc.scalar.activation(vt[:], vp[:], AF.Identity, bias=bv_sb[:, 0:1])
            tp = psA.tile([P, FD], f32, tag="a", name=f"tp_{qc}")
            for j in range(FD // P):
                nc.tensor.transpose(tp[:, ds(j * P, P)], vt[:, ds(j * P, P)],
                                    id_sb[:])
            nc.scalar.copy(v_sb[:, qc * (FD // P):(qc + 1) * (FD // P), :], tp[:])
            yield

        def pull(fillers, k):
            while k > 0 and fillers:
                try:
                    next(fillers[0])
                    k -= 1
                    fillers.rotate(-1)
                except StopIteration:
                    fillers.popleft()

        def attention(qc, fillers):
            """Causal attention for q chunk qc, both heads per kt step."""
            qt = qts.pop(qc)
            zt = psZ.tile([P, W2], f32, tag="z", name=f"zt_{qc}")
            esum = sb.tile([P, W2], bf16, tag="esum", name=f"esum_{qc}")
            kt_max = 4 * qc + 3
            for kt in range(kt_max + 1):
                o = max(0, kt * P - qc * FD)
                stm = psB.tile([P, W2], f32, tag="B", name=f"stm_{qc}_{kt}")
                nc.tensor.matmul(stm[:, o:FD], kt_sb[:, ds(kt * P, P)],
                                 qt[:, 0, o:FD], start=True, stop=True)
                nc.tensor.matmul(stm[:, FD + o:W2], kt_sb[:, ds(kt * P, P)],
                                 qt[:, 1, o:FD], start=True, stop=True)
                e = e_pool.tile([P, W2], bf16, tag="e", name=f"e_{qc}_{kt}")
                nc.scalar.activation(e[:, o:W2], stm[:, o:W2], AF.Exp,
                                     scale=1.0 / ATTN_SCALE)
                if kt >= 4 * qc:  # diagonal 128-block: causal mask inside
                    nc.vector.tensor_mul(e[:, o:o + P], e[:, o:o + P], mask_sb[:])
                    nc.vector.tensor_mul(e[:, FD + o:FD + o + P],
                                         e[:, FD + o:FD + o + P], mask_sb[:])
                if kt == 0:
                    nc.vector.tensor_copy(esum[:], e[:])
                elif o == 0:
                    nc.vector.tensor_add(esum[:], esum[:], e[:])
                else:
                    nc.vector.tensor_add(esum[:, o:FD], esum[:, o:FD],
                                         e[:, o:FD])
                    nc.vector.tensor_add(esum[:, FD + o:W2], esum[:, FD + o:W2],
                                         e[:, FD + o:W2])
                acc = dict(start=(kt == 0), stop=(kt == kt_max))
                nc.tensor.matmul(zt[:, o:FD], v_sb[:, kt, :], e[:, o:FD], **acc)
                nc.tensor.matmul(zt[:, FD + o:W2], v_sb[:, kt, :],
                                 e[:, FD + o:W2], **acc)
                pull(fillers, 2)
            # denominator: contract esum's 128 k-rows with a ones column
            den = psB.tile([P, W2], f32, tag="B", name=f"den_{qc}")
            nc.tensor.matmul(den[0:1, 0:FD], ones_sb[:, 0:1], esum[:, 0:FD],
                             start=True, stop=True)
            nc.tensor.matmul(den[0:1, FD:W2], ones_sb[:, 0:1], esum[:, FD:W2],
                             start=True, stop=True)
            pull(fillers, 3)
            rf = sb.tile([1, W2], f32, tag="rf", name=f"rf_{qc}")
            nc.vector.reciprocal_approx_fast(rf[:], den[0:1, :])
            rr = sb.tile([1, W2], bf16, tag="rr", name=f"rr_{qc}")
            nc.vector.tensor_scalar_mul(rr[:], rf[:], 1.0)
            rden_ps = psB.tile([P, W2], f32, tag="B", name=f"rden_{qc}")
            nc.tensor.matmul(rden_ps[:, 0:FD], ones_sb[0:1, :], rr[0:1, 0:FD],
                             start=True, stop=True)
            nc.tensor.matmul(rden_ps[:, FD:W2], ones_sb[0:1, :], rr[0:1, FD:W2],
                             start=True, stop=True)
            pull(fillers, len(fillers) + 40)  # drain remaining fillers
            rden = sb.tile([P, W2], f32, tag="rden", name=f"rd_{qc}")
            nc.vector.tensor_copy(rden[:], rden_ps[:])
            ztn = sb.tile([P, W2], bf16, tag="ztn", name=f"z_{qc}")
            nc.vector.tensor_mul(ztn[:], zt[:], rden[:])
            ztn_map[qc] = ztn

        def outproj_gen(qc):
            ztn = ztn_map.pop(qc)
            for sub in range(FD // P):
                for mcp in range(2):
                    op_ps = psB.tile([P, W2], f32, tag="B",
                                     name=f"op_{qc}_{sub}_{mcp}")
                    for half in range(2):
                        mc = mcp * 2 + half
                        for h in range(H):
                            nc.tensor.matmul(
                                op_ps[:, half * FD:(half + 1) * FD],
                                ztn[:, h * FD + sub * P:h * FD + sub * P + P],
                                wo_sb[:, h, ds(mc * FD, FD)],
                                start=(h == 0), stop=(h == H - 1))
                    ot = sb.tile([P, W2], bf16, tag="ot",
                                 name=f"ot_{qc}_{sub}_{mcp}")
                    if (sub + mcp) % 2 == 0:
                        nc.scalar.copy(ot[:], op_ps[:])
                    else:
                        nc.vector.tensor_copy(ot[:], op_ps[:])
                    nc.sync.dma_start(
                        out[ds(qc * FD + sub * P, P), ds(mcp * W2, W2)], ot[:])
                    yield

        # ---- schedule ----
        # x tiles are prefetched two chunks ahead of their projection matmuls
        # (one full attention window of DMA lead time). outproj(qc) is
        # emitted as filler work inside attention(qc+1) so its den-chain
        # dependency (DVE reciprocal tail) never idles the PE.
        load_x(0)
        for _ in projQ_gen(0):
            pass
        load_x(1)
        for _ in projKV_gen(0):
            pass
        load_x(2)
        for _ in projQ_gen(1):
            pass
        pending = None
        for qc in range(QC):
            if qc + 3 < QC:
                load_x(qc + 3)
            fillers = deque()
            if pending is not None:
                fillers.append(pending)
            if qc + 1 < QC:
                fillers.append(projKV_gen(qc + 1))
            if qc + 2 < QC:
                fillers.append(projQ_gen(qc + 2))
            attention(qc, fillers)
            pending = outproj_gen(qc)
        for _ in pending:
            pass
    nc.compile()
    return nc


_PERM = None


def _perm():
    global _PERM
    if _PERM is None:
        _PERM = np.concatenate([np.arange(0, D_HEAD, 2), np.arange(1, D_HEAD, 2)])
    return _PERM


def host_inputs(x, W_Q, W_K, W_V, W_O, b_Q, b_K, b_V, core,
                heads_per_core=2):
    """Build the per-core input map (numpy, named as in build_bass)."""
    import ml_dtypes
    bf16 = ml_dtypes.bfloat16
    seq = x.shape[0]
    perm = _perm()
    h0 = core * heads_per_core
    kv = h0 // (N_HEADS // N_KV)
    pairs = D_HEAD // 2
    freqs = 1.0 / ROTARY_BASE ** (np.arange(pairs, dtype=np.float64) / pairs)
    ang = np.outer(np.arange(seq), freqs)  # [seq, 64]
    cos = np.cos(ang).T.astype(np.float32)  # [64, seq]
    sin = np.sin(ang).T.astype(np.float32)
    return {
        "xT": np.ascontiguousarray(x.T).astype(bf16),
        "wq": np.ascontiguousarray(W_Q[h0:h0 + heads_per_core][:, :, perm]).astype(bf16),
        "wk": np.ascontiguousarray(W_K[kv][:, perm]).astype(bf16),
        "wv": np.ascontiguousarray(W_V[kv]).astype(bf16),
        "wo": np.ascontiguousarray(W_O[h0:h0 + heads_per_core]).astype(bf16),
        "bq": np.ascontiguousarray(
            b_Q[h0:h0 + heads_per_core][:, perm]
            .reshape(heads_per_core, 2, 64).transpose(2, 0, 1)),
        "bk": np.ascontiguousarray(b_K[kv][perm].reshape(2, 64).T),
        "bv": np.ascontiguousarray(b_V[kv][:, None]),
        "cos2": cos.astype(bf16),
        "sin2": sin.astype(bf16),
        "ident": np.eye(P, dtype=np.float32),
        "maskm": np.triu(np.ones((P, P), dtype=np.float32)).astype(bf16),
        "onesd": np.ones((P, P), dtype=np.float32).astype(bf16),
    }


_NC_CACHE = {}


def kernel(x, W_Q, W_K, W_V, W_O, b_Q, b_K, b_V, b_O):
    import sys
    if "/opt/trn_rl_repo" not in sys.path:
        sys.path.insert(0, "/opt/trn_rl_repo")
    from concourse import bass_utils

    x = np.asarray(x, dtype=np.float32)
    key = (x.shape[0], x.shape[1])
    if key not in _NC_CACHE:
        _NC_CACHE[key] = build_bass(seq=x.shape[0], d_model=x.shape[1])
    nc = _NC_CACHE[key]

    in_maps = [
        host_inputs(x, np.asarray(W_Q, np.float32), np.asarray(W_K, np.float32),
                    np.asarray(W_V, np.float32), np.asarray(W_O, np.float32),
                    np.asarray(b_Q, np.float32), np.asarray(b_K, np.float32),
                    np.asarray(b_V, np.float32), core)
        for core in range(N_CORES)
    ]
    res = bass_utils.run_bass_kernel_spmd(nc, in_maps, core_ids=list(range(N_CORES)))
    total = np.zeros((x.shape[0], x.shape[1]), dtype=np.float32)
    for r in res.results:
        total += np.asarray(r["out"], dtype=np.float32)
    total += np.asarray(b_O, np.float32)[None, :]
    return total


# revision 30
# speedup vs baseline: 1.5985x; 1.1084x over previous
"""Trainium2 Bass kernel for causal GQA attention (nn_Attention_83090437308676).

Full shapes: x [4096, 2048], 16 Q heads / 4 KV heads, d_head=128, fp32, causal,
rotary (interleaved pairs, rotary_dim=128), out = attn @ W_O + b_O.

Sharding: tensor-parallel over heads. Core c computes Q-heads {2c, 2c+1} and
KV-head c//2 (duplicated across the pair of cores sharing it), produces the
partial output z_h @ W_O_h summed over its 2 heads; the host sums the 8
partials (bf16) in fp32 and adds b_O.

v2 vs baseline (615us):
 - all matmul operands bf16 (fp32 PSUM accumulation stays) -> ~2x PE rate; the
   HW runs f32r matmuls at ~1.3GHz effective vs ~2.4GHz for bf16.
 - denominator: instead of a ones-stationary matmul per (head, kt) tile (a
   full extra pass of e through the PE), e tiles are accumulated on the DVE
   into a per-chunk esum [128, 2*FD]; one ones-matmul per (chunk, head)
   contracts the final 128 k-rows.
 - both heads merged per kt step: scores land in one [128, 1024] PSUM tile
   (2 banks), a single Exp instruction evacuates both heads (halves the Act
   engine's fixed per-instruction overhead).
 - explicit software pipelining: K/V projection of chunk qc+1 and Q projection
   of chunk qc+2 are emitted as "filler" slices between attention kt steps so
   the PE never waits on the softmax (Act) chain; x tiles for chunk qc+2 are
   DMA-prefetched at the start of attention(qc).
"""

from collections import deque

import numpy as np

SEQ = 4096
D_MODEL = 2048
D_HEAD = 128
N_HEADS = 16
N_KV = 4
N_CORES = 8
ROTARY_BASE = 10000.0
ATTN_SCALE = 11.313708498984761  # sqrt(d_head)

P = 128  # partitions
FD = 512  # matmul moving free dim / chunk width


def build_bass(seq=SEQ, d_model=D_MODEL, heads_per_core=2):
    """Emit the per-core Tile kernel. Same program for all cores (SPMD);
    per-core tensors differ only in data."""
    from contextlib import ExitStack

    import concourse.mybir as mybir
    import concourse.tile as tile
    from concourse import bacc
    from concourse.bass import ds

    f32 = mybir.dt.float32
    bf16 = mybir.dt.bfloat16
    AF = mybir.ActivationFunctionType
    OP = mybir.AluOpType

    H = heads_per_core
    DM_TILES = d_model // P      # contraction tiles for projections
    QC = seq // FD               # 512-wide seq chunks
    W2 = 2 * FD                  # merged two-head tile width

    nc = bacc.Bacc("TRN2", target_bir_lowering=False, debug=False,
                   num_devices=N_CORES)

    # All weights / x are host-prepacked partition-major so each loads with a
    # single fully-contiguous DMA (the SP engine issues 2D DMAs at ~600ns
    # each -- many small transfers would serialize the prologue).
    xp = nc.dram_tensor("xp", (P, d_model // P, seq), bf16,
                        kind="ExternalInput").ap()
    wq = nc.dram_tensor("wq", (P, H, d_model // P, D_HEAD), bf16,
                        kind="ExternalInput").ap()
    wk = nc.dram_tensor("wk", (P, d_model // P, D_HEAD), bf16,
                        kind="ExternalInput").ap()
    wv = nc.dram_tensor("wv", (P, d_model // P, D_HEAD), bf16,
                        kind="ExternalInput").ap()
    wo = nc.dram_tensor("wo", (P, H, d_model), bf16, kind="ExternalInput").ap()
    bq = nc.dram_tensor("bq", (64, H, 2), f32, kind="ExternalInput").ap()
    bk = nc.dram_tensor("bk", (64, 2), f32, kind="ExternalInput").ap()
    bv = nc.dram_tensor("bv", (P, 1), f32, kind="ExternalInput").ap()
    cos2 = nc.dram_tensor("cos2", (64, seq), bf16, kind="ExternalInput").ap()
    sin2 = nc.dram_tensor("sin2", (64, seq), bf16, kind="ExternalInput").ap()
    ident = nc.dram_tensor("ident", (P, P), f32, kind="ExternalInput").ap()
    maskm = nc.dram_tensor("maskm", (P, P), bf16, kind="ExternalInput").ap()
    onesd = nc.dram_tensor("onesd", (P, P), bf16, kind="ExternalInput").ap()
    out = nc.dram_tensor("out", (seq, d_model), bf16, kind="ExternalOutput").ap()

    with tile.TileContext(nc) as tc, ExitStack() as ctx:
        const = ctx.enter_context(tc.tile_pool(name="const", bufs=1))
        persist = ctx.enter_context(tc.tile_pool(name="persist", bufs=1))
        xt_pool = ctx.enter_context(tc.tile_pool(name="xt", bufs=14))
        qt_pool = ctx.enter_context(tc.tile_pool(name="qt", bufs=3))
        e_pool = ctx.enter_context(tc.tile_pool(name="e", bufs=3))
        sb = ctx.enter_context(tc.tile_pool(name="sb", bufs=2))
        # PSUM: big pool = 2 x [128,1024] (4 banks): stm / den / rden / op
        #       zt pool  = 1 x [128,1024] (2 banks): per-chunk PV accumulator
        #       acc pool = 2 x [128,512]  (2 banks): qp pair / kp+vp / tp
        psB = ctx.enter_context(tc.tile_pool(name="psB", bufs=2, space="PSUM"))
        psZ = ctx.enter_context(tc.tile_pool(name="psZ", bufs=1, space="PSUM"))
        psA = ctx.enter_context(tc.tile_pool(name="psA", bufs=2, space="PSUM"))

        # ---- constants / weights resident in SBUF ----
        # cos/sin first: the chunk-0 rotary (critical path to the first
        # attention matmul) waits on them.
        cos_sb = const.tile([64, seq], bf16, tag="cos")
        sin_sb = const.tile([64, seq], bf16, tag="sin")
        nc.sync.dma_start(cos_sb[:], cos2)
        nc.sync.dma_start(sin_sb[:], sin2)
        wq_sb = const.tile([P, H, DM_TILES, D_HEAD], bf16, tag="wq")
        nc.sync.dma_start(wq_sb[:], wq)
        wk_sb = const.tile([P, DM_TILES, D_HEAD], bf16, tag="wk")
        nc.sync.dma_start(wk_sb[:], wk)
        wv_sb = const.tile([P, DM_TILES, D_HEAD], bf16, tag="wv")
        nc.sync.dma_start(wv_sb[:], wv)
        id_sb = const.tile([P, P], f32, tag="id")
        nc.sync.dma_start(id_sb[:], ident)
        mask_sb = const.tile([P, P], bf16, tag="mask")
        nc.sync.dma_start(mask_sb[:], maskm)
        bq_sb = const.tile([64, H, 2], f32, tag="bq")
        nc.sync.dma_start(bq_sb[:], bq)
        bk_sb = const.tile([64, 2], f32, tag="bk")
        nc.sync.dma_start(bk_sb[:], bk)
        bv_sb = const.tile([P, 1], f32, tag="bv")
        nc.sync.dma_start(bv_sb[:], bv)
        ones_sb = const.tile([P, P], bf16, tag="ones")
        nc.sync.dma_start(ones_sb[:], onesd)
        wo_sb = const.tile([P, H, d_model], bf16, tag="wo")

        # K^T (rotated) and V (natural [k, d]) for this core's KV head.
        kt_sb = persist.tile([P, seq], bf16, tag="kt")
        v_sb = persist.tile([P, seq // P, P], bf16, tag="v")

        qts = {}       # qc -> qt tile
        xts_map = {}   # qc -> list of xt tiles (DMA prefetched)
        ztn_map = {}   # qc -> normalized z (bf16, [P, W2])

        def rotary_evac(psum, dst, b_ap, qc):
            """dst ([P, FD] slice, bf16) = rotary(psum + bias) at chunk qc.

            The bias add + bf16 cast goes through the Act engine (per-partition
            bias); the cos/sin algebra then runs all-bf16 on the DVE at double
            rate (the [64, x] ops only use half the lanes, so halving the
            element cost matters)."""
            sl = ds(qc * FD, FD)
            x1 = sb.tile([64, FD], bf16, tag="qsb1")
            x2 = sb.tile([64, FD], bf16, tag="qsb2")
            nc.scalar.activation(x1[:], psum[0:64, :], AF.Identity,
                                 bias=b_ap[:, 0:1])
            nc.scalar.activation(x2[:], psum[64:128, :], AF.Identity,
                                 bias=b_ap[:, 1:2])
            x1, x2 = x1[:], x2[:]
            t1 = sb.tile([64, FD], bf16, tag="rot_t1")
            t2 = sb.tile([64, FD], bf16, tag="rot_t2")
            t3 = sb.tile([64, FD], bf16, tag="rot_t3")
            t4 = sb.tile([64, FD], bf16, tag="rot_t4")
            nc.vector.tensor_mul(t1[:], x1, cos_sb[:, sl])
            nc.vector.tensor_mul(t2[:], x2, sin_sb[:, sl])
            nc.vector.tensor_mul(t3[:], x1, sin_sb[:, sl])
            nc.vector.tensor_mul(t4[:], x2, cos_sb[:, sl])
            nc.vector.tensor_sub(dst[0:64, :], t1[:], t2[:])
            nc.vector.tensor_add(dst[64:128, :], t3[:], t4[:])

        def load_x(qc):
            """DMA-prefetch the x tiles for chunk qc (4 groups of 4 d-tiles)."""
            xts = [xt_pool.tile([P, 4, FD], bf16, tag="xt", name=f"xt_{qc}_{g}")
                   for g in range(4)]
            for g in range(4):
                nc.sync.dma_start(xts[g][:],
                                  xp[:, 4 * g:4 * g + 4, ds(qc * FD, FD)])
            xts_map[qc] = xts

        def projQ_gen(qc):
            """Q projection + rotary for chunk qc (xts already prefetched)."""
            xts = xts_map[qc]
            qp = [psA.tile([P, FD], f32, tag="a", name=f"qp{h}_{qc}")
                  for h in range(H)]
            for t in range(DM_TILES):
                xt_ap = xts[t // 4][:, t % 4, :]
                mm = dict(start=(t == 0), stop=(t == DM_TILES - 1))
                for h in range(H):
                    nc.tensor.matmul(qp[h][:], wq_sb[:, h, t, :], xt_ap, **mm)
                yield
            qt = qt_pool.tile([P, H, FD], bf16, tag="qt", name=f"qt_{qc}")
            for h in range(H):
                rotary_evac(qp[h], qt[:, h, :], bq_sb[:, h, :], qc)
            qts[qc] = qt
            yield

        def projKV_gen(qc):
            """K/V projection for chunk qc: K rotary -> kt_sb, V -> v_sb."""
            xts = xts_map[qc]
            kp = psA.tile([P, FD], f32, tag="a", name=f"kp_{qc}")
            vp = psA.tile([P, FD], f32, tag="a", name=f"vp_{qc}")
            for t in range(DM_TILES):
                xt_ap = xts[t // 4][:, t % 4, :]
                mm = dict(start=(t == 0), stop=(t == DM_TILES - 1))
                nc.tensor.matmul(kp[:], wk_sb[:, t, :], xt_ap, **mm)
                nc.tensor.matmul(vp[:], wv_sb[:, t, :], xt_ap, **mm)
                yield
            if qc == 0:
                nc.sync.dma_start(wo_sb[:], wo)
            rotary_evac(kp, kt_sb[:, ds(qc * FD, FD)], bk_sb, qc)
            yield
            # V: bias add then transpose to natural [k, d] layout (f32 through
            # the PE transpose; cast to bf16 on the PSUM->v_sb copy)
            vt = sb.tile([P, FD], f32, tag="vt")
            nc.scalar.activation(vt[:], vp[:], AF.Identity, bias=bv_sb[:, 0:1])
            tp = psA.tile([P, FD], f32, tag="a", name=f"tp_{qc}")
            for j in range(FD // P):
                nc.tensor.transpose(tp[:, ds(j * P, P)], vt[:, ds(j * P, P)],
                                    id_sb[:])
            nc.scalar.copy(v_sb[:, qc * (FD // P):(qc + 1) * (FD // P), :], tp[:])
            yield

        def pull(fillers, k):
            while k > 0 and fillers:
                try:
                    next(fillers[0])
                    k -= 1
                    fillers.rotate(-1)
                except StopIteration:
                    fillers.popleft()

        def attention(qc, fillers):
            """Causal attention for q chunk qc, both heads per kt step."""
            qt = qts.pop(qc)
            zt = psZ.tile([P, W2], f32, tag="z", name=f"zt_{qc}")
            esum = sb.tile([P, W2], bf16, tag="esum", name=f"esum_{qc}")
            kt_max = 4 * qc + 3
            for kt in range(kt_max + 1):
                o = max(0, kt * P - qc * FD)
                stm = psB.tile([P, W2], f32, tag="B", name=f"stm_{qc}_{kt}")
                nc.tensor.matmul(stm[:, o:FD], kt_sb[:, ds(kt * P, P)],
                                 qt[:, 0, o:FD], start=True, stop=True)
                nc.tensor.matmul(stm[:, FD + o:W2], kt_sb[:, ds(kt * P, P)],
                                 qt[:, 1, o:FD], start=True, stop=True)
                e = e_pool.tile([P, W2], bf16, tag="e", name=f"e_{qc}_{kt}")
                nc.scalar.activation(e[:, o:W2], stm[:, o:W2], AF.Exp,
                                     scale=1.0 / ATTN_SCALE)
                if kt >= 4 * qc:  # diagonal 128-block: causal mask inside
                    nc.vector.tensor_mul(e[:, o:o + P], e[:, o:o + P], mask_sb[:])
                    nc.vector.tensor_mul(e[:, FD + o:FD + o + P],
                                         e[:, FD + o:FD + o + P], mask_sb[:])
                if kt == 0:
                    nc.vector.tensor_copy(esum[:], e[:])
                elif o == 0:
                    nc.vector.tensor_add(esum[:], esum[:], e[:])
                else:
                    nc.vector.tensor_add(esum[:, o:FD], esum[:, o:FD],
                                         e[:, o:FD])
                    nc.vector.tensor_add(esum[:, FD + o:W2], esum[:, FD + o:W2],
                                         e[:, FD + o:W2])
                acc = dict(start=(kt == 0), stop=(kt == kt_max))
                nc.tensor.matmul(zt[:, o:FD], v_sb[:, kt, :], e[:, o:FD], **acc)
                nc.tensor.matmul(zt[:, FD + o:W2], v_sb[:, kt, :],
                                 e[:, FD + o:W2], **acc)
                pull(fillers, 2)
            # denominator: contract esum's 128 k-rows with a ones column
            den = psB.tile([P, W2], f32, tag="B", name=f"den_{qc}")
            nc.tensor.matmul(den[0:1, 0:FD], ones_sb[:, 0:1], esum[:, 0:FD],
                             start=True, stop=True)
            nc.tensor.matmul(den[0:1, FD:W2], ones_sb[:, 0:1], esum[:, FD:W2],
                             start=True, stop=True)
            pull(fillers, 3)
            rf = sb.tile([1, W2], f32, tag="rf", name=f"rf_{qc}")
            nc.vector.reciprocal_approx_fast(rf[:], den[0:1, :])
            rr = sb.tile([1, W2], bf16, tag="rr", name=f"rr_{qc}")
            nc.vector.tensor_scalar_mul(rr[:], rf[:], 1.0)
            rden_ps = psB.tile([P, W2], f32, tag="B", name=f"rden_{qc}")
            nc.tensor.matmul(rden_ps[:, 0:FD], ones_sb[0:1, :], rr[0:1, 0:FD],
                             start=True, stop=True)
            nc.tensor.matmul(rden_ps[:, FD:W2], ones_sb[0:1, :], rr[0:1, FD:W2],
                             start=True, stop=True)
            pull(fillers, len(fillers) + 40)  # drain remaining fillers
            rden = sb.tile([P, W2], f32, tag="rden", name=f"rd_{qc}")
            nc.vector.tensor_copy(rden[:], rden_ps[:])
            ztn = sb.tile([P, W2], bf16, tag="ztn", name=f"z_{qc}")
            nc.vector.tensor_mul(ztn[:], zt[:], rden[:])
            ztn_map[qc] = ztn

        def outproj_gen(qc):
            ztn = ztn_map.pop(qc)
            for sub in range(FD // P):
                ot = sb.tile([P, 2 * W2], bf16, tag="ot",
                             name=f"ot_{qc}_{sub}")
                for mcp in range(2):
                    op_ps = psB.tile([P, W2], f32, tag="B",
                                     name=f"op_{qc}_{sub}_{mcp}")
                    for half in range(2):
                        mc = mcp * 2 + half
                        for h in range(H):
                            nc.tensor.matmul(
                                op_ps[:, half * FD:(half + 1) * FD],
                                ztn[:, h * FD + sub * P:h * FD + sub * P + P],
                                wo_sb[:, h, ds(mc * FD, FD)],
                                start=(h == 0), stop=(h == H - 1))
                    if mcp == 0:
                        nc.scalar.copy(ot[:, 0:W2], op_ps[:])
                    else:
                        nc.vector.tensor_copy(ot[:, W2:2 * W2], op_ps[:])
                    yield
                nc.sync.dma_start(out[ds(qc * FD + sub * P, P), :], ot[:])

        # ---- schedule ----
        # x tiles are prefetched two chunks ahead of their projection matmuls
        # (one full attention window of DMA lead time). outproj(qc) is
        # emitted as filler work inside attention(qc+1) so its den-chain
        # dependency (DVE reciprocal tail) never idles the PE.
        load_x(0)
        for _ in projQ_gen(0):
            pass
        load_x(1)
        for _ in projKV_gen(0):
            pass
        load_x(2)
        for _ in projQ_gen(1):
            pass
        pending = None
        for qc in range(QC):
            if qc + 3 < QC:
                load_x(qc + 3)
            fillers = deque()
            if pending is not None:
                fillers.append(pending)
            if qc + 1 < QC:
                fillers.append(projKV_gen(qc + 1))
            if qc + 2 < QC:
                fillers.append(projQ_gen(qc + 2))
            attention(qc, fillers)
            pending = outproj_gen(qc)
        for _ in pending:
            pass
    nc.compile()
    return nc


_PERM = None


def _perm():
    global _PERM
    if _PERM is None:
        _PERM = np.concatenate([np.arange(0, D_HEAD, 2), np.arange(1, D_HEAD, 2)])
    return _PERM


def host_inputs(x, W_Q, W_K, W_V, W_O, b_Q, b_K, b_V, core,
                heads_per_core=2):
    """Build the per-core input map (numpy, named as in build_bass)."""
    import ml_dtypes
    bf16 = ml_dtypes.bfloat16
    seq = x.shape[0]
    perm = _perm()
    h0 = core * heads_per_core
    kv = h0 // (N_HEADS // N_KV)
    pairs = D_HEAD // 2
    freqs = 1.0 / ROTARY_BASE ** (np.arange(pairs, dtype=np.float64) / pairs)
    ang = np.outer(np.arange(seq), freqs)  # [seq, 64]
    cos = np.cos(ang).T.astype(np.float32)  # [64, seq]
    sin = np.sin(ang).T.astype(np.float32)
    dm_t = x.shape[1] // P
    return {
        # x^T prepacked partition-major: xp[p, t, s] = x[s, t*128 + p]
        "xp": np.ascontiguousarray(
            x.T.reshape(dm_t, P, seq).transpose(1, 0, 2)).astype(bf16),
        "wq": np.ascontiguousarray(
            W_Q[h0:h0 + heads_per_core][:, :, perm]
            .reshape(heads_per_core, dm_t, P, D_HEAD)
            .transpose(2, 0, 1, 3)).astype(bf16),
        "wk": np.ascontiguousarray(
            W_K[kv][:, perm].reshape(dm_t, P, D_HEAD)
            .transpose(1, 0, 2)).astype(bf16),
        "wv": np.ascontiguousarray(
            W_V[kv].reshape(dm_t, P, D_HEAD).transpose(1, 0, 2)).astype(bf16),
        "wo": np.ascontiguousarray(
            W_O[h0:h0 + heads_per_core].transpose(1, 0, 2)).astype(bf16),
        "bq": np.ascontiguousarray(
            b_Q[h0:h0 + heads_per_core][:, perm]
            .reshape(heads_per_core, 2, 64).transpose(2, 0, 1)),
        "bk": np.ascontiguousarray(b_K[kv][perm].reshape(2, 64).T),
        "bv": np.ascontiguousarray(b_V[kv][:, None]),
        "cos2": cos.astype(bf16),
        "sin2": sin.astype(bf16),
        "ident": np.eye(P, dtype=np.float32),
        "maskm": np.triu(np.ones((P, P), dtype=np.float32)).astype(bf16),
        "onesd": np.ones((P, P), dtype=np.float32).astype(bf16),
    }


_NC_CACHE = {}


def kernel(x, W_Q, W_K, W_V, W_O, b_Q, b_K, b_V, b_O):
    import sys
    if "/opt/trn_rl_repo" not in sys.path:
        sys.path.insert(0, "/opt/trn_rl_repo")
    from concourse import bass_utils

    x = np.asarray(x, dtype=np.float32)
    key = (x.shape[0], x.shape[1])
    if key not in _NC_CACHE:
        _NC_CACHE[key] = build_bass(seq=x.shape[0], d_model=x.shape[1])
    nc = _NC_CACHE[key]

    in_maps = [
        host_inputs(x, np.asarray(W_Q, np.float32), np.asarray(W_K, np.float32),
                    np.asarray(W_V, np.float32), np.asarray(W_O, np.float32),
                    np.asarray(b_Q, np.float32), np.asarray(b_K, np.float32),
                    np.asarray(b_V, np.float32), core)
        for core in range(N_CORES)
    ]
    res = bass_utils.run_bass_kernel_spmd(nc, in_maps, core_ids=list(range(N_CORES)))
    total = np.zeros((x.shape[0], x.shape[1]), dtype=np.float32)
    for r in res.results:
        total += np.asarray(r["out"], dtype=np.float32)
    total += np.asarray(b_O, np.float32)[None, :]
    return total


# revision 65
# speedup vs baseline: 1.6292x; 1.0192x over previous
"""Trainium2 Bass kernel for causal GQA attention (nn_Attention_83090437308676).

Full shapes: x [4096, 2048], 16 Q heads / 4 KV heads, d_head=128, fp32, causal,
rotary (interleaved pairs, rotary_dim=128), out = attn @ W_O + b_O.

Sharding: tensor-parallel over heads. Core c computes Q-heads {2c, 2c+1} and
KV-head c//2 (duplicated across the pair of cores sharing it), produces the
partial output z_h @ W_O_h summed over its 2 heads; the host sums the 8
partials (bf16) in fp32 and adds b_O.

Optimizations vs the 615us f32r baseline (~378us measured):
 - all matmul operands bf16 (fp32 PSUM accumulation stays): HW streams f32r
   matmuls at ~1.3GHz effective vs full 2.4GHz (216ns/512-wide) for bf16.
 - denominator: instead of a ones-stationary matmul per (head, kt) tile (a
   full extra e pass through the PE), e tiles accumulate on the DVE into a
   per-chunk esum [128, 2*FD]; one ones-matmul per (chunk, head) contracts
   the final 128 k-rows. 1/den is broadcast across partitions on the
   otherwise-idle GpSimd engine, per head-half so outproj's h0 matmuls start
   after half the chain.
 - both heads merged per kt step: scores land in one [128, 1024] PSUM tile
   (2 banks), a single Exp instruction evacuates both heads (halves the Act
   engine's fixed ~190ns per-instruction overhead).
 - explicit software pipelining: K/V projection of chunk qc+1 + Q projection
   of chunk qc+2 are "must" filler slices between attention kt steps and
   outproj(qc-1..qc-2) is spillable filler, so the PE never waits on the
   softmax (Act) chain; x tiles are DMA-prefetched two chunks ahead.
 - host prepacks x/weights partition-major so every load is one contiguous
   DMA (the SP engine issues 2D DMAs at ~600ns each); output rows leave as
   single fully-contiguous [128, 4KB] DMAs.
 - DMA priority order with wide margins: biases/cos/sin first (a tight
   margin intermittently let the first rotary read cos_sb before the DMA
   landed -> chunk-0 NaN), then wq/x(0) interleaved in quarter slices.
"""

from collections import deque

import numpy as np

SEQ = 4096
D_MODEL = 2048
D_HEAD = 128
N_HEADS = 16
N_KV = 4
N_CORES = 8
ROTARY_BASE = 10000.0
ATTN_SCALE = 11.313708498984761  # sqrt(d_head)

P = 128  # partitions
FD = 512  # matmul moving free dim / chunk width


def build_bass(seq=SEQ, d_model=D_MODEL, heads_per_core=2):
    """Emit the per-core Tile kernel. Same program for all cores (SPMD);
    per-core tensors differ only in data."""
    from contextlib import ExitStack

    import concourse.mybir as mybir
    import concourse.tile as tile
    from concourse import bacc
    from concourse.bass import ds

    f32 = mybir.dt.float32
    bf16 = mybir.dt.bfloat16
    AF = mybir.ActivationFunctionType
    OP = mybir.AluOpType

    H = heads_per_core
    DM_TILES = d_model // P      # contraction tiles for projections
    QC = seq // FD               # 512-wide seq chunks
    W2 = 2 * FD                  # merged two-head tile width

    nc = bacc.Bacc("TRN2", target_bir_lowering=False, debug=False,
                   num_devices=N_CORES)

    # All weights / x are host-prepacked partition-major so each loads with a
    # single fully-contiguous DMA (the SP engine issues 2D DMAs at ~600ns
    # each -- many small transfers would serialize the prologue).
    xp = nc.dram_tensor("xp", (P, d_model // P, seq), bf16,
                        kind="ExternalInput").ap()
    wq = nc.dram_tensor("wq", (P, H, d_model // P, D_HEAD), bf16,
                        kind="ExternalInput").ap()
    wk = nc.dram_tensor("wk", (P, d_model // P, D_HEAD), bf16,
                        kind="ExternalInput").ap()
    wv = nc.dram_tensor("wv", (P, d_model // P, D_HEAD), bf16,
                        kind="ExternalInput").ap()
    wo = nc.dram_tensor("wo", (P, H, d_model), bf16, kind="ExternalInput").ap()
    bq = nc.dram_tensor("bq", (64, H, 2), f32, kind="ExternalInput").ap()
    bk = nc.dram_tensor("bk", (64, 2), f32, kind="ExternalInput").ap()
    bv = nc.dram_tensor("bv", (P, 1), f32, kind="ExternalInput").ap()
    cos2 = nc.dram_tensor("cos2", (64, seq), bf16, kind="ExternalInput").ap()
    sin2 = nc.dram_tensor("sin2", (64, seq), bf16, kind="ExternalInput").ap()
    ident = nc.dram_tensor("ident", (P, P), f32, kind="ExternalInput").ap()
    maskm = nc.dram_tensor("maskm", (P, P), bf16, kind="ExternalInput").ap()
    onesd = nc.dram_tensor("onesd", (P, P), bf16, kind="ExternalInput").ap()
    out = nc.dram_tensor("out", (seq, d_model), bf16, kind="ExternalOutput").ap()

    with tile.TileContext(nc) as tc, ExitStack() as ctx:
        const = ctx.enter_context(tc.tile_pool(name="const", bufs=1))
        persist = ctx.enter_context(tc.tile_pool(name="persist", bufs=1))
        xt_pool = ctx.enter_context(tc.tile_pool(name="xt", bufs=14))
        qt_pool = ctx.enter_context(tc.tile_pool(name="qt", bufs=3))
        e_pool = ctx.enter_context(tc.tile_pool(name="e", bufs=3))
        sb = ctx.enter_context(tc.tile_pool(name="sb", bufs=2))
        # PSUM: big pool = 2 x [128,1024] (4 banks): stm / den / rden / op
        #       zt pool  = 1 x [128,1024] (2 banks): per-chunk PV accumulator
        #       acc pool = 2 x [128,512]  (2 banks): qp pair / kp+vp / tp
        psB = ctx.enter_context(tc.tile_pool(name="psB", bufs=2, space="PSUM"))
        psZ = ctx.enter_context(tc.tile_pool(name="psZ", bufs=1, space="PSUM"))
        psA = ctx.enter_context(tc.tile_pool(name="psA", bufs=2, space="PSUM"))

        # ---- constants / weights resident in SBUF ----
        # DMA priority with generous margins (a tight margin intermittently
        # let the first rotary read cos_sb before its DMA landed -> chunk-0
        # NaN): biases + cos/sin (first use ~18us) lead, then wq + x(0)
        # (first matmul), then the remaining consts (first use >=25us).
        bq_sb = const.tile([64, H, 2], f32, tag="bq")
        nc.sync.dma_start(bq_sb[:], bq)
        bk_sb = const.tile([64, 2], f32, tag="bk")
        nc.sync.dma_start(bk_sb[:], bk)
        bv_sb = const.tile([P, 1], f32, tag="bv")
        nc.sync.dma_start(bv_sb[:], bv)
        cos_sb = const.tile([64, seq], bf16, tag="cos")
        nc.sync.dma_start(cos_sb[:], cos2)
        sin_sb = const.tile([64, seq], bf16, tag="sin")
        nc.sync.dma_start(sin_sb[:], sin2)
        wq_sb = const.tile([P, H, DM_TILES, D_HEAD], bf16, tag="wq")
        wk_sb = const.tile([P, DM_TILES, D_HEAD], bf16, tag="wk")
        wv_sb = const.tile([P, DM_TILES, D_HEAD], bf16, tag="wv")
        mask_sb = const.tile([P, P], bf16, tag="mask")
        ones_sb = const.tile([P, P], bf16, tag="ones")
        id_sb = const.tile([P, P], f32, tag="id")
        wo_sb = const.tile([P, H, d_model], bf16, tag="wo")

        def load_consts():
            nc.sync.dma_start(mask_sb[:], maskm)
            nc.sync.dma_start(ones_sb[:], onesd)
            nc.sync.dma_start(id_sb[:], ident)

        # K^T (rotated) and V (natural [k, d]) for this core's KV head.
        kt_sb = persist.tile([P, seq], bf16, tag="kt")
        v_sb = persist.tile([P, seq // P, P], bf16, tag="v")

        qts = {}       # qc -> qt tile
        xts_map = {}   # qc -> list of xt tiles (DMA prefetched)
        ztn_map = {}   # qc -> normalized z (bf16, [P, W2])
        in_chain = [False]  # True while the den-chain needs the DVE clear

        def rotary_evac(psum, dst, b_ap, qc):
            """dst ([P, FD] slice, bf16) = rotary(psum + bias) at chunk qc.

            The bias add + bf16 cast goes through the Act engine (per-partition
            bias); the cos/sin algebra then runs all-bf16 on the DVE at double
            rate (the [64, x] ops only use half the lanes, so halving the
            element cost matters)."""
            sl = ds(qc * FD, FD)
            x1 = sb.tile([64, FD], bf16, tag="qsb1")
            x2 = sb.tile([64, FD], bf16, tag="qsb2")
            nc.scalar.activation(x1[:], psum[0:64, :], AF.Identity,
                                 bias=b_ap[:, 0:1])
            nc.scalar.activation(x2[:], psum[64:128, :], AF.Identity,
                                 bias=b_ap[:, 1:2])
            x1, x2 = x1[:], x2[:]
            t1 = sb.tile([64, FD], bf16, tag="rot_t1")
            t2 = sb.tile([64, FD], bf16, tag="rot_t2")
            t3 = sb.tile([64, FD], bf16, tag="rot_t3")
            t4 = sb.tile([64, FD], bf16, tag="rot_t4")
            nc.vector.tensor_mul(t1[:], x1, cos_sb[:, sl])
            nc.vector.tensor_mul(t2[:], x2, sin_sb[:, sl])
            nc.vector.tensor_mul(t3[:], x1, sin_sb[:, sl])
            nc.vector.tensor_mul(t4[:], x2, cos_sb[:, sl])
            nc.vector.tensor_sub(dst[0:64, :], t1[:], t2[:])
            nc.vector.tensor_add(dst[64:128, :], t3[:], t4[:])

        def load_x(qc):
            """DMA-prefetch the x tiles for chunk qc (4 groups of 4 d-tiles)."""
            xts = [xt_pool.tile([P, 4, FD], bf16, tag="xt", name=f"xt_{qc}_{g}")
                   for g in range(4)]
            for g in range(4):
                nc.sync.dma_start(xts[g][:],
                                  xp[:, 4 * g:4 * g + 4, ds(qc * FD, FD)])
            xts_map[qc] = xts

        def projQ_gen(qc):
            """Q projection + rotary for chunk qc (xts already prefetched)."""
            xts = xts_map[qc]
            qp = [psA.tile([P, FD], f32, tag="a", name=f"qp{h}_{qc}")
                  for h in range(H)]
            for t in range(DM_TILES):
                xt_ap = xts[t // 4][:, t % 4, :]
                mm = dict(start=(t == 0), stop=(t == DM_TILES - 1))
                for h in range(H):
                    nc.tensor.matmul(qp[h][:], wq_sb[:, h, t, :], xt_ap, **mm)
                yield
            qt = qt_pool.tile([P, H, FD], bf16, tag="qt", name=f"qt_{qc}")
            for h in range(H):
                rotary_evac(qp[h], qt[:, h, :], bq_sb[:, h, :], qc)
            qts[qc] = qt
            yield

        def projKV_gen(qc):
            """K/V projection for chunk qc: K rotary -> kt_sb, V -> v_sb."""
            xts = xts_map[qc]
            kp = psA.tile([P, FD], f32, tag="a", name=f"kp_{qc}")
            vp = psA.tile([P, FD], f32, tag="a", name=f"vp_{qc}")
            for t in range(DM_TILES):
                xt_ap = xts[t // 4][:, t % 4, :]
                mm = dict(start=(t == 0), stop=(t == DM_TILES - 1))
                nc.tensor.matmul(kp[:], wk_sb[:, t, :], xt_ap, **mm)
                nc.tensor.matmul(vp[:], wv_sb[:, t, :], xt_ap, **mm)
                yield
            if qc == 0:
                nc.sync.dma_start(wo_sb[:], wo)
            rotary_evac(kp, kt_sb[:, ds(qc * FD, FD)], bk_sb, qc)
            yield
            # V: bias add then transpose to natural [k, d] layout (f32 through
            # the PE transpose; cast to bf16 on the PSUM->v_sb copy)
            vt = sb.tile([P, FD], f32, tag="vt")
            nc.scalar.activation(vt[:], vp[:], AF.Identity, bias=bv_sb[:, 0:1])
            tp = psA.tile([P, FD], f32, tag="a", name=f"tp_{qc}")
            for j in range(FD // P):
                nc.tensor.transpose(tp[:, ds(j * P, P)], vt[:, ds(j * P, P)],
                                    id_sb[:])
            nc.scalar.copy(v_sb[:, qc * (FD // P):(qc + 1) * (FD // P), :], tp[:])
            yield

        def pull(dq, k, seq_order=False):
            while k > 0 and dq:
                try:
                    next(dq[0][2])
                    k -= 1
                    if not seq_order:
                        dq.rotate(-1)
                except StopIteration:
                    dq.popleft()

        def drain(dq, pred=lambda tag, qq: True):
            keep = deque()
            while dq:
                tag, qq, g = dq.popleft()
                if pred(tag, qq):
                    for _ in g:
                        pass
                else:
                    keep.append((tag, qq, g))
            dq.extend(keep)

        def attention(qc, must, spill):
            """Causal attention for q chunk qc, both heads per kt step."""
            qt = qts.pop(qc)
            zt = psZ.tile([P, W2], f32, tag="z", name=f"zt_{qc}")
            esum = sb.tile([P, W2], bf16, tag="esum", name=f"esum_{qc}")
            kt_max = 4 * qc + 3
            for kt in range(kt_max + 1):
                o = max(0, kt * P - qc * FD)
                stm = psB.tile([P, W2], f32, tag="B", name=f"stm_{qc}_{kt}")
                nc.tensor.matmul(stm[:, o:FD], kt_sb[:, ds(kt * P, P)],
                                 qt[:, 0, o:FD], start=True, stop=True)
                nc.tensor.matmul(stm[:, FD + o:W2], kt_sb[:, ds(kt * P, P)],
                                 qt[:, 1, o:FD], start=True, stop=True)
                e = e_pool.tile([P, W2], bf16, tag="e", name=f"e_{qc}_{kt}")
                nc.scalar.activation(e[:, o:W2], stm[:, o:W2], AF.Exp,
                                     scale=1.0 / ATTN_SCALE)
                if kt >= 4 * qc:  # diagonal 128-block: causal mask inside
                    nc.vector.tensor_mul(e[:, o:o + P], e[:, o:o + P], mask_sb[:])
                    nc.vector.tensor_mul(e[:, FD + o:FD + o + P],
                                         e[:, FD + o:FD + o + P], mask_sb[:])
                if kt == 0:
                    nc.vector.tensor_copy(esum[:], e[:])
                elif o == 0:
                    nc.vector.tensor_add(esum[:], esum[:], e[:])
                else:
                    nc.vector.tensor_add(esum[:, o:FD], esum[:, o:FD],
                                         e[:, o:FD])
                    nc.vector.tensor_add(esum[:, FD + o:W2], esum[:, FD + o:W2],
                                         e[:, FD + o:W2])
                acc = dict(start=(kt == 0), stop=(kt == kt_max))
                nc.tensor.matmul(zt[:, o:FD], v_sb[:, kt, :], e[:, o:FD], **acc)
                nc.tensor.matmul(zt[:, FD + o:W2], v_sb[:, kt, :],
                                 e[:, FD + o:W2], **acc)
                # must is drained strictly in order: K/V of qc+1 before Q of
                # qc+2 (they share the psA accumulator ring with the next
                # chunk's K/V -- interleaving across chunks would race).
                # Late chunks have little projection work left; pull spill
                # slower there so deferred outproj remains to cover the
                # den-chain latency.
                pull(must, 2, seq_order=True)
                # ration spill (deferred outproj) so ~8 units remain to cover
                # each den-chain; late chunks are supply-starved on top.
                # (Banking more units for late windows backfires: the den-
                # chain force-drain dumps them with Act-routed evacuations
                # that then block the next chunk's exp stream.)
                if qc < 6:
                    pull(spill, 1)
                elif kt % (2 if qc == 6 else 4) == 0:
                    pull(spill, 1)
            # K/V of chunk qc+1 must be fully emitted before attention(qc+1)
            # reads it; old outproj must finish before its ztn slot recycles.
            drain(must)
            den = psB.tile([P, W2], f32, tag="B", name=f"den_{qc}")
            nc.tensor.matmul(den[0:1, 0:FD], ones_sb[:, 0:1], esum[:, 0:FD],
                             start=True, stop=True)
            nc.tensor.matmul(den[0:1, FD:W2], ones_sb[:, 0:1], esum[:, FD:W2],
                             start=True, stop=True)
            in_chain[0] = True
            pull(spill, 2)
            # Per-head-half normalization chain so outproj's h0 matmuls can
            # start after ~recip+bcast+mul of half 0 instead of the full W2
            # chain: DVE recip -> GpSimd partition-broadcast -> DVE multiply.
            rf = sb.tile([1, W2], f32, tag="rf", name=f"rf_{qc}")
            nc.vector.reciprocal_approx_fast(rf[0:1, 0:FD], den[0:1, 0:FD])
            nc.vector.reciprocal_approx_fast(rf[0:1, FD:W2], den[0:1, FD:W2])
            rden = sb.tile([P, W2], f32, tag="rden", name=f"rd_{qc}")
            nc.gpsimd.partition_broadcast(rden[:, 0:FD], rf[0:1, 0:FD],
                                          channels=P)
            nc.gpsimd.partition_broadcast(rden[:, FD:W2], rf[0:1, FD:W2],
                                          channels=P)
            ztn = sb.tile([P, W2], bf16, tag="ztn", bufs=3, name=f"z_{qc}")
            nc.vector.tensor_mul(ztn[:, 0:FD], zt[:, 0:FD], rden[:, 0:FD])
            nc.vector.tensor_mul(ztn[:, FD:W2], zt[:, FD:W2], rden[:, FD:W2])
            pull(spill, 6)
            drain(spill, lambda tag, qq: tag == "op" and qq <= qc - 2)
            in_chain[0] = False
            ztn_map[qc] = ztn

        def outproj_gen(qc):
            ztn = ztn_map.pop(qc)
            for sub in range(FD // P):
                ot = sb.tile([P, 2 * W2], bf16, tag="ot",
                             name=f"ot_{qc}_{sub}")
                for mcp in range(2):
                    op_ps = psB.tile([P, W2], f32, tag="B",
                                     name=f"op_{qc}_{sub}_{mcp}")
                    # h-outer so the h0 matmuls only depend on ztn's first
                    # half (the den-chain normalizes half 0 first)
                    for h in range(H):
                        for half in range(2):
                            mc = mcp * 2 + half
                            nc.tensor.matmul(
                                op_ps[:, half * FD:(half + 1) * FD],
                                ztn[:, h * FD + sub * P:h * FD + sub * P + P],
                                wo_sb[:, h, ds(mc * FD, FD)],
                                start=(h == 0), stop=(h == H - 1))
                    # during the den-chain keep the DVE clear (its queue
                    # delays the chain); otherwise alternate Act/DVE
                    if in_chain[0] or mcp == 0:
                        nc.scalar.copy(ot[:, mcp * W2:(mcp + 1) * W2], op_ps[:])
                    else:
                        nc.vector.tensor_copy(ot[:, W2:2 * W2], op_ps[:])
                    yield
                nc.sync.dma_start(out[ds(qc * FD + sub * P, P), :], ot[:])

        # ---- schedule ----
        # x tiles are prefetched two chunks ahead of their projection matmuls
        # (one full attention window of DMA lead time). Filler work between
        # attention kt steps: K/V projection of qc+1 ("must" -- drained by
        # chunk end), plus a spillable pool (Q projection of qc+2, deferred
        # outproj of earlier chunks) that carries forward so the early
        # chunks' surplus PE work fills the late chunks' exp-gated windows.
        # wq and x(0) interleaved in 4 t-group slices so the first projection
        # matmul waits only on the first quarter of each
        xts0 = [xt_pool.tile([P, 4, FD], bf16, tag="xt", name=f"xt_0_{g}")
                for g in range(4)]
        for g in range(4):
            nc.sync.dma_start(wq_sb[:, :, 4 * g:4 * g + 4, :],
                              wq[:, :, 4 * g:4 * g + 4, :])
            nc.sync.dma_start(xts0[g][:], xp[:, 4 * g:4 * g + 4, ds(0, FD)])
        xts_map[0] = xts0
        nc.sync.dma_start(wk_sb[:], wk)
        nc.sync.dma_start(wv_sb[:], wv)
        for _ in projQ_gen(0):
            pass
        load_consts()
        load_x(1)
        for _ in projKV_gen(0):
            pass
        load_x(2)
        spill = deque()
        for qc in range(QC):
            if qc + 3 < QC:
                load_x(qc + 3)
            must = deque()
            if qc == 0:
                must.append(("q", 1, projQ_gen(1)))
            if qc + 1 < QC:
                must.append(("kv", qc + 1, projKV_gen(qc + 1)))
            if qc + 2 < QC:
                must.append(("q", qc + 2, projQ_gen(qc + 2)))
            attention(qc, must, spill)
            spill.append(("op", qc, outproj_gen(qc)))
        drain(spill)
    nc.compile()
    return nc


_PERM = None


def _perm():
    global _PERM
    if _PERM is None:
        _PERM = np.concatenate([np.arange(0, D_HEAD, 2), np.arange(1, D_HEAD, 2)])
    return _PERM


def host_inputs(x, W_Q, W_K, W_V, W_O, b_Q, b_K, b_V, core,
                heads_per_core=2):
    """Build the per-core input map (numpy, named as in build_bass)."""
    import ml_dtypes
    bf16 = ml_dtypes.bfloat16
    seq = x.shape[0]
    perm = _perm()
    h0 = core * heads_per_core
    kv = h0 // (N_HEADS // N_KV)
    pairs = D_HEAD // 2
    freqs = 1.0 / ROTARY_BASE ** (np.arange(pairs, dtype=np.float64) / pairs)
    ang = np.outer(np.arange(seq), freqs)  # [seq, 64]
    cos = np.cos(ang).T.astype(np.float32)  # [64, seq]
    sin = np.sin(ang).T.astype(np.float32)
    dm_t = x.shape[1] // P
    return {
        # x^T prepacked partition-major: xp[p, t, s] = x[s, t*128 + p]
        "xp": np.ascontiguousarray(
            x.T.reshape(dm_t, P, seq).transpose(1, 0, 2)).astype(bf16),
        "wq": np.ascontiguousarray(
            W_Q[h0:h0 + heads_per_core][:, :, perm]
            .reshape(heads_per_core, dm_t, P, D_HEAD)
            .transpose(2, 0, 1, 3)).astype(bf16),
        "wk": np.ascontiguousarray(
            W_K[kv][:, perm].reshape(dm_t, P, D_HEAD)
            .transpose(1, 0, 2)).astype(bf16),
        "wv": np.ascontiguousarray(
            W_V[kv].reshape(dm_t, P, D_HEAD).transpose(1, 0, 2)).astype(bf16),
        "wo": np.ascontiguousarray(
            W_O[h0:h0 + heads_per_core].transpose(1, 0, 2)).astype(bf16),
        "bq": np.ascontiguousarray(
            b_Q[h0:h0 + heads_per_core][:, perm]
            .reshape(heads_per_core, 2, 64).transpose(2, 0, 1)),
        "bk": np.ascontiguousarray(b_K[kv][perm].reshape(2, 64).T),
        "bv": np.ascontiguousarray(b_V[kv][:, None]),
        "cos2": cos.astype(bf16),
        "sin2": sin.astype(bf16),
        "ident": np.eye(P, dtype=np.float32),
        "maskm": np.triu(np.ones((P, P), dtype=np.float32)).astype(bf16),
        "onesd": np.ones((P, P), dtype=np.float32).astype(bf16),
    }


_NC_CACHE = {}


def kernel(x, W_Q, W_K, W_V, W_O, b_Q, b_K, b_V, b_O):
    import sys
    if "/opt/trn_rl_repo" not in sys.path:
        sys.path.insert(0, "/opt/trn_rl_repo")
    from concourse import bass_utils

    x = np.asarray(x, dtype=np.float32)
    key = (x.shape[0], x.shape[1])
    if key not in _NC_CACHE:
        _NC_CACHE[key] = build_bass(seq=x.shape[0], d_model=x.shape[1])
    nc = _NC_CACHE[key]

    in_maps = [
        host_inputs(x, np.asarray(W_Q, np.float32), np.asarray(W_K, np.float32),
                    np.asarray(W_V, np.float32), np.asarray(W_O, np.float32),
                    np.asarray(b_Q, np.float32), np.asarray(b_K, np.float32),
                    np.asarray(b_V, np.float32), core)
        for core in range(N_CORES)
    ]
    res = bass_utils.run_bass_kernel_spmd(nc, in_maps, core_ids=list(range(N_CORES)))
    total = np.zeros((x.shape[0], x.shape[1]), dtype=np.float32)
    for r in res.results:
        total += np.asarray(r["out"], dtype=np.float32)
    total += np.asarray(b_O, np.float32)[None, :]
    return total


# revision 67
# speedup vs baseline: 1.6367x; 1.0046x over previous
"""Trainium2 Bass kernel for causal GQA attention (nn_Attention_83090437308676).

Full shapes: x [4096, 2048], 16 Q heads / 4 KV heads, d_head=128, fp32, causal,
rotary (interleaved pairs, rotary_dim=128), out = attn @ W_O + b_O.

Sharding: tensor-parallel over heads. Core c computes Q-heads {2c, 2c+1} and
KV-head c//2 (duplicated across the pair of cores sharing it), produces the
partial output z_h @ W_O_h summed over its 2 heads; the host sums the 8
partials (bf16) in fp32 and adds b_O.

Optimizations vs the 615us f32r baseline (~378us measured):
 - all matmul operands bf16 (fp32 PSUM accumulation stays): HW streams f32r
   matmuls at ~1.3GHz effective vs full 2.4GHz (216ns/512-wide) for bf16.
 - denominator: instead of a ones-stationary matmul per (head, kt) tile (a
   full extra e pass through the PE), e tiles accumulate on the DVE into a
   per-chunk esum [128, 2*FD]; one ones-matmul per (chunk, head) contracts
   the final 128 k-rows. 1/den is broadcast across partitions on the
   otherwise-idle GpSimd engine, per head-half so outproj's h0 matmuls start
   after half the chain.
 - both heads merged per kt step: scores land in one [128, 1024] PSUM tile
   (2 banks), a single Exp instruction evacuates both heads (halves the Act
   engine's fixed ~190ns per-instruction overhead).
 - explicit software pipelining: K/V projection of chunk qc+1 + Q projection
   of chunk qc+2 are "must" filler slices between attention kt steps and
   outproj(qc-1..qc-2) is spillable filler, so the PE never waits on the
   softmax (Act) chain; x tiles are DMA-prefetched two chunks ahead.
 - host prepacks x/weights partition-major so every load is one contiguous
   DMA (the SP engine issues 2D DMAs at ~600ns each); output rows leave as
   single fully-contiguous [128, 4KB] DMAs.
 - DMA priority order with wide margins: biases/cos/sin first (a tight
   margin intermittently let the first rotary read cos_sb before the DMA
   landed -> chunk-0 NaN), then wq/x(0) interleaved in quarter slices.
"""

from collections import deque

import numpy as np

SEQ = 4096
D_MODEL = 2048
D_HEAD = 128
N_HEADS = 16
N_KV = 4
N_CORES = 8
ROTARY_BASE = 10000.0
ATTN_SCALE = 11.313708498984761  # sqrt(d_head)

P = 128  # partitions
FD = 512  # matmul moving free dim / chunk width


def build_bass(seq=SEQ, d_model=D_MODEL, heads_per_core=2):
    """Emit the per-core Tile kernel. Same program for all cores (SPMD);
    per-core tensors differ only in data."""
    from contextlib import ExitStack

    import concourse.mybir as mybir
    import concourse.tile as tile
    from concourse import bacc
    from concourse.bass import ds

    f32 = mybir.dt.float32
    bf16 = mybir.dt.bfloat16
    AF = mybir.ActivationFunctionType
    OP = mybir.AluOpType

    H = heads_per_core
    DM_TILES = d_model // P      # contraction tiles for projections
    QC = seq // FD               # 512-wide seq chunks
    W2 = 2 * FD                  # merged two-head tile width

    nc = bacc.Bacc("TRN2", target_bir_lowering=False, debug=False,
                   num_devices=N_CORES)

    # All weights / x are host-prepacked partition-major so each loads with a
    # single fully-contiguous DMA (the SP engine issues 2D DMAs at ~600ns
    # each -- many small transfers would serialize the prologue).
    xp = nc.dram_tensor("xp", (P, d_model // P, seq), bf16,
                        kind="ExternalInput").ap()
    wq = nc.dram_tensor("wq", (P, H, d_model // P, D_HEAD), bf16,
                        kind="ExternalInput").ap()
    wk = nc.dram_tensor("wk", (P, d_model // P, D_HEAD), bf16,
                        kind="ExternalInput").ap()
    wv = nc.dram_tensor("wv", (P, d_model // P, D_HEAD), bf16,
                        kind="ExternalInput").ap()
    wo = nc.dram_tensor("wo", (P, H, d_model), bf16, kind="ExternalInput").ap()
    bq = nc.dram_tensor("bq", (64, H, 2), f32, kind="ExternalInput").ap()
    bk = nc.dram_tensor("bk", (64, 2), f32, kind="ExternalInput").ap()
    bv = nc.dram_tensor("bv", (P, 1), f32, kind="ExternalInput").ap()
    cos2 = nc.dram_tensor("cos2", (64, seq), bf16, kind="ExternalInput").ap()
    sin2 = nc.dram_tensor("sin2", (64, seq), bf16, kind="ExternalInput").ap()
    ident = nc.dram_tensor("ident", (P, P), f32, kind="ExternalInput").ap()
    maskm = nc.dram_tensor("maskm", (P, P), bf16, kind="ExternalInput").ap()
    onesd = nc.dram_tensor("onesd", (P, P), bf16, kind="ExternalInput").ap()
    out = nc.dram_tensor("out", (seq, d_model), bf16, kind="ExternalOutput").ap()

    with tile.TileContext(nc) as tc, ExitStack() as ctx:
        const = ctx.enter_context(tc.tile_pool(name="const", bufs=1))
        persist = ctx.enter_context(tc.tile_pool(name="persist", bufs=1))
        xt_pool = ctx.enter_context(tc.tile_pool(name="xt", bufs=14))
        qt_pool = ctx.enter_context(tc.tile_pool(name="qt", bufs=3))
        e_pool = ctx.enter_context(tc.tile_pool(name="e", bufs=3))
        sb = ctx.enter_context(tc.tile_pool(name="sb", bufs=2))
        # PSUM: big pool = 2 x [128,1024] (4 banks): stm / den / rden / op
        #       zt pool  = 1 x [128,1024] (2 banks): per-chunk PV accumulator
        #       acc pool = 2 x [128,512]  (2 banks): qp pair / kp+vp / tp
        psB = ctx.enter_context(tc.tile_pool(name="psB", bufs=2, space="PSUM"))
        psZ = ctx.enter_context(tc.tile_pool(name="psZ", bufs=1, space="PSUM"))
        psA = ctx.enter_context(tc.tile_pool(name="psA", bufs=2, space="PSUM"))

        # ---- constants / weights resident in SBUF ----
        # DMA priority with generous margins (a tight margin intermittently
        # let the first rotary read cos_sb before its DMA landed -> chunk-0
        # NaN): biases + cos/sin (first use ~18us) lead, then wq + x(0)
        # (first matmul), then the remaining consts (first use >=25us).
        bq_sb = const.tile([64, H, 2], f32, tag="bq")
        nc.sync.dma_start(bq_sb[:], bq)
        bk_sb = const.tile([64, 2], f32, tag="bk")
        nc.sync.dma_start(bk_sb[:], bk)
        bv_sb = const.tile([P, 1], f32, tag="bv")
        nc.sync.dma_start(bv_sb[:], bv)
        cos_sb = const.tile([64, seq], bf16, tag="cos")
        nc.sync.dma_start(cos_sb[:], cos2)
        sin_sb = const.tile([64, seq], bf16, tag="sin")
        nc.sync.dma_start(sin_sb[:], sin2)
        wq_sb = const.tile([P, H, DM_TILES, D_HEAD], bf16, tag="wq")
        wk_sb = const.tile([P, DM_TILES, D_HEAD], bf16, tag="wk")
        wv_sb = const.tile([P, DM_TILES, D_HEAD], bf16, tag="wv")
        mask_sb = const.tile([P, P], bf16, tag="mask")
        ones_sb = const.tile([P, P], bf16, tag="ones")
        id_sb = const.tile([P, P], f32, tag="id")
        wo_sb = const.tile([P, H, d_model], bf16, tag="wo")

        def load_consts():
            nc.sync.dma_start(mask_sb[:], maskm)
            nc.sync.dma_start(ones_sb[:], onesd)
            nc.sync.dma_start(id_sb[:], ident)

        # K^T (rotated) and V (natural [k, d]) for this core's KV head.
        kt_sb = persist.tile([P, seq], bf16, tag="kt")
        v_sb = persist.tile([P, seq // P, P], bf16, tag="v")

        qts = {}       # qc -> qt tile
        xts_map = {}   # qc -> list of xt tiles (DMA prefetched)
        ztn_map = {}   # qc -> normalized z (bf16, [P, W2])
        in_chain = [False]  # True while the den-chain needs the DVE clear

        def rotary_evac(psum, dst, b_ap, qc):
            """dst ([P, FD] slice, bf16) = rotary(psum + bias) at chunk qc.

            The bias add + bf16 cast goes through the Act engine (per-partition
            bias); the cos/sin algebra then runs all-bf16 on the DVE at double
            rate (the [64, x] ops only use half the lanes, so halving the
            element cost matters)."""
            sl = ds(qc * FD, FD)
            x1 = sb.tile([64, FD], bf16, tag="qsb1")
            x2 = sb.tile([64, FD], bf16, tag="qsb2")
            nc.scalar.activation(x1[:], psum[0:64, :], AF.Identity,
                                 bias=b_ap[:, 0:1])
            nc.scalar.activation(x2[:], psum[64:128, :], AF.Identity,
                                 bias=b_ap[:, 1:2])
            x1, x2 = x1[:], x2[:]
            t1 = sb.tile([64, FD], bf16, tag="rot_t1")
            t2 = sb.tile([64, FD], bf16, tag="rot_t2")
            t3 = sb.tile([64, FD], bf16, tag="rot_t3")
            t4 = sb.tile([64, FD], bf16, tag="rot_t4")
            nc.vector.tensor_mul(t1[:], x1, cos_sb[:, sl])
            nc.vector.tensor_mul(t2[:], x2, sin_sb[:, sl])
            nc.vector.tensor_mul(t3[:], x1, sin_sb[:, sl])
            nc.vector.tensor_mul(t4[:], x2, cos_sb[:, sl])
            nc.vector.tensor_sub(dst[0:64, :], t1[:], t2[:])
            nc.vector.tensor_add(dst[64:128, :], t3[:], t4[:])

        def load_x(qc):
            """DMA-prefetch the x tiles for chunk qc (4 groups of 4 d-tiles)."""
            xts = [xt_pool.tile([P, 4, FD], bf16, tag="xt", name=f"xt_{qc}_{g}")
                   for g in range(4)]
            for g in range(4):
                nc.sync.dma_start(xts[g][:],
                                  xp[:, 4 * g:4 * g + 4, ds(qc * FD, FD)])
            xts_map[qc] = xts

        def projQ_gen(qc):
            """Q projection + rotary for chunk qc (xts already prefetched)."""
            xts = xts_map[qc]
            qp = [psA.tile([P, FD], f32, tag="a", name=f"qp{h}_{qc}")
                  for h in range(H)]
            for t in range(DM_TILES):
                xt_ap = xts[t // 4][:, t % 4, :]
                mm = dict(start=(t == 0), stop=(t == DM_TILES - 1))
                for h in range(H):
                    nc.tensor.matmul(qp[h][:], wq_sb[:, h, t, :], xt_ap, **mm)
                yield
            qt = qt_pool.tile([P, H, FD], bf16, tag="qt", name=f"qt_{qc}")
            for h in range(H):
                rotary_evac(qp[h], qt[:, h, :], bq_sb[:, h, :], qc)
            qts[qc] = qt
            yield

        def projKV_gen(qc):
            """K/V projection for chunk qc: K rotary -> kt_sb, V -> v_sb."""
            xts = xts_map[qc]
            kp = psA.tile([P, FD], f32, tag="a", name=f"kp_{qc}")
            vp = psA.tile([P, FD], f32, tag="a", name=f"vp_{qc}")
            for t in range(DM_TILES):
                xt_ap = xts[t // 4][:, t % 4, :]
                mm = dict(start=(t == 0), stop=(t == DM_TILES - 1))
                nc.tensor.matmul(kp[:], wk_sb[:, t, :], xt_ap, **mm)
                nc.tensor.matmul(vp[:], wv_sb[:, t, :], xt_ap, **mm)
                yield
            if qc == 0:
                nc.sync.dma_start(wo_sb[:], wo)
            rotary_evac(kp, kt_sb[:, ds(qc * FD, FD)], bk_sb, qc)
            yield
            # V: bias add then transpose to natural [k, d] layout (f32 through
            # the PE transpose; cast to bf16 on the PSUM->v_sb copy)
            vt = sb.tile([P, FD], f32, tag="vt")
            nc.scalar.activation(vt[:], vp[:], AF.Identity, bias=bv_sb[:, 0:1])
            tp = psA.tile([P, FD], f32, tag="a", name=f"tp_{qc}")
            for j in range(FD // P):
                nc.tensor.transpose(tp[:, ds(j * P, P)], vt[:, ds(j * P, P)],
                                    id_sb[:])
            nc.scalar.copy(v_sb[:, qc * (FD // P):(qc + 1) * (FD // P), :], tp[:])
            yield

        def pull(dq, k, seq_order=False):
            while k > 0 and dq:
                try:
                    next(dq[0][2])
                    k -= 1
                    if not seq_order:
                        dq.rotate(-1)
                except StopIteration:
                    dq.popleft()

        def drain(dq, pred=lambda tag, qq: True):
            keep = deque()
            while dq:
                tag, qq, g = dq.popleft()
                if pred(tag, qq):
                    for _ in g:
                        pass
                else:
                    keep.append((tag, qq, g))
            dq.extend(keep)

        def attention(qc, must, spill):
            """Causal attention for q chunk qc, both heads per kt step."""
            qt = qts.pop(qc)
            zt = psZ.tile([P, W2], f32, tag="z", name=f"zt_{qc}")
            esum = sb.tile([P, W2], bf16, tag="esum", name=f"esum_{qc}")
            kt_max = 4 * qc + 3
            for kt in range(kt_max + 1):
                o = max(0, kt * P - qc * FD)
                stm = psB.tile([P, W2], f32, tag="B", name=f"stm_{qc}_{kt}")
                nc.tensor.matmul(stm[:, o:FD], kt_sb[:, ds(kt * P, P)],
                                 qt[:, 0, o:FD], start=True, stop=True)
                nc.tensor.matmul(stm[:, FD + o:W2], kt_sb[:, ds(kt * P, P)],
                                 qt[:, 1, o:FD], start=True, stop=True)
                e = e_pool.tile([P, W2], bf16, tag="e", name=f"e_{qc}_{kt}")
                nc.scalar.activation(e[:, o:W2], stm[:, o:W2], AF.Exp,
                                     scale=1.0 / ATTN_SCALE)
                if kt >= 4 * qc:  # diagonal 128-block: causal mask inside
                    nc.vector.tensor_mul(e[:, o:o + P], e[:, o:o + P], mask_sb[:])
                    nc.vector.tensor_mul(e[:, FD + o:FD + o + P],
                                         e[:, FD + o:FD + o + P], mask_sb[:])
                if kt == 0:
                    nc.vector.tensor_copy(esum[:], e[:])
                elif o == 0:
                    nc.vector.tensor_add(esum[:], esum[:], e[:])
                else:
                    nc.vector.tensor_add(esum[:, o:FD], esum[:, o:FD],
                                         e[:, o:FD])
                    nc.vector.tensor_add(esum[:, FD + o:W2], esum[:, FD + o:W2],
                                         e[:, FD + o:W2])
                acc = dict(start=(kt == 0), stop=(kt == kt_max))
                nc.tensor.matmul(zt[:, o:FD], v_sb[:, kt, :], e[:, o:FD], **acc)
                nc.tensor.matmul(zt[:, FD + o:W2], v_sb[:, kt, :],
                                 e[:, FD + o:W2], **acc)
                # must is drained strictly in order: K/V of qc+1 before Q of
                # qc+2 (they share the psA accumulator ring with the next
                # chunk's K/V -- interleaving across chunks would race).
                # Late chunks have little projection work left; pull spill
                # slower there so deferred outproj remains to cover the
                # den-chain latency.
                pull(must, 2, seq_order=True)
                # ration spill (deferred outproj) so ~8 units remain to cover
                # each den-chain; late chunks are supply-starved on top.
                # (Banking more units for late windows backfires: the den-
                # chain force-drain dumps them with Act-routed evacuations
                # that then block the next chunk's exp stream.)
                if qc < 6:
                    pull(spill, 1)
                elif kt % (2 if qc == 6 else 4) == 0:
                    pull(spill, 1)
            # K/V of chunk qc+1 must be fully emitted before attention(qc+1)
            # reads it; old outproj must finish before its ztn slot recycles.
            drain(must)
            den = psB.tile([P, W2], f32, tag="B", name=f"den_{qc}")
            nc.tensor.matmul(den[0:1, 0:FD], ones_sb[:, 0:1], esum[:, 0:FD],
                             start=True, stop=True)
            nc.tensor.matmul(den[0:1, FD:W2], ones_sb[:, 0:1], esum[:, FD:W2],
                             start=True, stop=True)
            in_chain[0] = True
            pull(spill, 2)
            # Per-head-half normalization chain so outproj's h0 matmuls can
            # start after ~recip+bcast+mul of half 0 instead of the full W2
            # chain: DVE recip -> GpSimd partition-broadcast -> DVE multiply.
            rf = sb.tile([1, W2], f32, tag="rf", name=f"rf_{qc}")
            nc.vector.reciprocal_approx_fast(rf[0:1, 0:FD], den[0:1, 0:FD])
            nc.vector.reciprocal_approx_fast(rf[0:1, FD:W2], den[0:1, FD:W2])
            rden = sb.tile([P, W2], f32, tag="rden", name=f"rd_{qc}")
            nc.gpsimd.partition_broadcast(rden[:, 0:FD], rf[0:1, 0:FD],
                                          channels=P)
            nc.gpsimd.partition_broadcast(rden[:, FD:W2], rf[0:1, FD:W2],
                                          channels=P)
            ztn = sb.tile([P, W2], bf16, tag="ztn", bufs=3, name=f"z_{qc}")
            nc.vector.tensor_mul(ztn[:, 0:FD], zt[:, 0:FD], rden[:, 0:FD])
            nc.vector.tensor_mul(ztn[:, FD:W2], zt[:, FD:W2], rden[:, FD:W2])
            pull(spill, 6)
            drain(spill, lambda tag, qq: tag == "op" and qq <= qc - 2)
            in_chain[0] = False
            ztn_map[qc] = ztn

        def outproj_gen(qc):
            ztn = ztn_map.pop(qc)
            for sub in range(FD // P):
                ot = sb.tile([P, 2 * W2], bf16, tag="ot",
                             name=f"ot_{qc}_{sub}")
                for mcp in range(2):
                    op_ps = psB.tile([P, W2], f32, tag="B",
                                     name=f"op_{qc}_{sub}_{mcp}")
                    # h-outer so the h0 matmuls only depend on ztn's first
                    # half (the den-chain normalizes half 0 first)
                    for h in range(H):
                        for half in range(2):
                            mc = mcp * 2 + half
                            nc.tensor.matmul(
                                op_ps[:, half * FD:(half + 1) * FD],
                                ztn[:, h * FD + sub * P:h * FD + sub * P + P],
                                wo_sb[:, h, ds(mc * FD, FD)],
                                start=(h == 0), stop=(h == H - 1))
                    # during the den-chain keep the DVE clear (its queue
                    # delays the chain); otherwise alternate Act/DVE
                    if in_chain[0] or mcp == 0:
                        nc.scalar.copy(ot[:, mcp * W2:(mcp + 1) * W2], op_ps[:])
                    else:
                        nc.vector.tensor_copy(ot[:, W2:2 * W2], op_ps[:])
                    yield
                nc.sync.dma_start(out[ds(qc * FD + sub * P, P), :], ot[:])

        # ---- schedule ----
        # x tiles are prefetched two chunks ahead of their projection matmuls
        # (one full attention window of DMA lead time). Filler work between
        # attention kt steps: K/V projection of qc+1 ("must" -- drained by
        # chunk end), plus a spillable pool (Q projection of qc+2, deferred
        # outproj of earlier chunks) that carries forward so the early
        # chunks' surplus PE work fills the late chunks' exp-gated windows.
        # wq and x(0) interleaved in 4 t-group slices so the first projection
        # matmul waits only on the first quarter of each
        xts0 = [xt_pool.tile([P, 4, FD], bf16, tag="xt", name=f"xt_0_{g}")
                for g in range(4)]
        for g in range(4):
            nc.sync.dma_start(wq_sb[:, :, 4 * g:4 * g + 4, :],
                              wq[:, :, 4 * g:4 * g + 4, :])
            nc.sync.dma_start(xts0[g][:], xp[:, 4 * g:4 * g + 4, ds(0, FD)])
        xts_map[0] = xts0
        nc.sync.dma_start(wk_sb[:], wk)
        nc.sync.dma_start(wv_sb[:], wv)
        for _ in projQ_gen(0):
            pass
        load_consts()
        load_x(1)
        for _ in projKV_gen(0):
            pass
        load_x(2)
        spill = deque()
        for qc in range(QC):
            if qc + 3 < QC:
                load_x(qc + 3)
            must = deque()
            if qc == 0:
                must.append(("q", 1, projQ_gen(1)))
            if qc + 1 < QC:
                must.append(("kv", qc + 1, projKV_gen(qc + 1)))
            if qc + 2 < QC:
                must.append(("q", qc + 2, projQ_gen(qc + 2)))
            attention(qc, must, spill)
            spill.append(("op", qc, outproj_gen(qc)))
        drain(spill)
    nc.compile()
    return nc


_PERM = None


def _perm():
    global _PERM
    if _PERM is None:
        _PERM = np.concatenate([np.arange(0, D_HEAD, 2), np.arange(1, D_HEAD, 2)])
    return _PERM


def host_inputs(x, W_Q, W_K, W_V, W_O, b_Q, b_K, b_V, core,
                heads_per_core=2):
    """Build the per-core input map (numpy, named as in build_bass)."""
    import ml_dtypes
    bf16 = ml_dtypes.bfloat16
    seq = x.shape[0]
    perm = _perm()
    h0 = core * heads_per_core
    kv = h0 // (N_HEADS // N_KV)
    pairs = D_HEAD // 2
    freqs = 1.0 / ROTARY_BASE ** (np.arange(pairs, dtype=np.float64) / pairs)
    ang = np.outer(np.arange(seq), freqs)  # [seq, 64]
    cos = np.cos(ang).T.astype(np.float32)  # [64, seq]
    sin = np.sin(ang).T.astype(np.float32)
    dm_t = x.shape[1] // P
    return {
        # x^T prepacked partition-major: xp[p, t, s] = x[s, t*128 + p]
        "xp": np.ascontiguousarray(
            x.T.reshape(dm_t, P, seq).transpose(1, 0, 2)).astype(bf16),
        "wq": np.ascontiguousarray(
            W_Q[h0:h0 + heads_per_core][:, :, perm]
            .reshape(heads_per_core, dm_t, P, D_HEAD)
            .transpose(2, 0, 1, 3)).astype(bf16),
        "wk": np.ascontiguousarray(
            W_K[kv][:, perm].reshape(dm_t, P, D_HEAD)
            .transpose(1, 0, 2)).astype(bf16),
        "wv": np.ascontiguousarray(
            W_V[kv].reshape(dm_t, P, D_HEAD).transpose(1, 0, 2)).astype(bf16),
        "wo": np.ascontiguousarray(
            W_O[h0:h0 + heads_per_core].transpose(1, 0, 2)).astype(bf16),
        "bq": np.ascontiguousarray(
            b_Q[h0:h0 + heads_per_core][:, perm]
            .reshape(heads_per_core, 2, 64).transpose(2, 0, 1)),
        "bk": np.ascontiguousarray(b_K[kv][perm].reshape(2, 64).T),
        "bv": np.ascontiguousarray(b_V[kv][:, None]),
        "cos2": cos.astype(bf16),
        "sin2": sin.astype(bf16),
        "ident": np.eye(P, dtype=np.float32),
        "maskm": np.triu(np.ones((P, P), dtype=np.float32)).astype(bf16),
        "onesd": np.ones((P, P), dtype=np.float32).astype(bf16),
    }


_NC_CACHE = {}


def kernel(x, W_Q, W_K, W_V, W_O, b_Q, b_K, b_V, b_O):
    import sys
    if "/opt/trn_rl_repo" not in sys.path:
        sys.path.insert(0, "/opt/trn_rl_repo")
    from concourse import bass_utils

    x = np.asarray(x, dtype=np.float32)
    key = (x.shape[0], x.shape[1])
    if key not in _NC_CACHE:
        _NC_CACHE[key] = build_bass(seq=x.shape[0], d_model=x.shape[1])
    nc = _NC_CACHE[key]

    in_maps = [
        host_inputs(x, np.asarray(W_Q, np.float32), np.asarray(W_K, np.float32),
                    np.asarray(W_V, np.float32), np.asarray(W_O, np.float32),
                    np.asarray(b_Q, np.float32), np.asarray(b_K, np.float32),
                    np.asarray(b_V, np.float32), core)
        for core in range(N_CORES)
    ]
    res = bass_utils.run_bass_kernel_spmd(nc, in_maps, core_ids=list(range(N_CORES)))
    total = np.zeros((x.shape[0], x.shape[1]), dtype=np.float32)
    for r in res.results:
        total += np.asarray(r["out"], dtype=np.float32)
    total += np.asarray(b_O, np.float32)[None, :]
    return total
